# revision 1
# baseline (speedup 1.0000x reference)
"""Multi-head attention (B=2, S=2048, D=1024, H=16, dk=64) on 8 Trainium2
NeuronCores via Bass/Tile.

Sharding: core c handles batch b = c//4 and head-group g = c%4 (4 heads,
256 qkv columns).  Each core computes its QKV projection slices, 4 heads of
attention, and a partial output projection against its 256-row slice of Wo.
The host sums the 4 partial outputs per batch (row-sharded Wo => partial
sums) and folds in the biases bo and bv@Wo (softmax rows sum to 1, so the
V-bias contributes exactly bv@Wo per token).

v2 design notes:
- All matmuls in float32r (full-rate fp32 PE mode, ~1.5e-4 rounding); PE
  transposes also f32r (1.5 cyc/row) to avoid dtype switches.
- scoresT [k_tok, q_tok] per head via K=64 row-packed head pairs
  (tile_position (0,0)/(64,0) derived from base partitions) -> concurrent.
- Scores land in one shared 4-bank PSUM tensor [128, 8, 512]; ONE ACT exp
  per 2 k-chunks covers [128, 2048] (amortizes the 352-cycle ACT overhead).
- AV lhsT = [1 | V_h] so PSUM row 0 accumulates the softmax denominators.
- Normalization without PE transposes: DVE reciprocal of the sums row,
  PE ones-outer-product broadcast to [65, 512], DVE multiply, then a
  partition-shifting SBUF->SBUF DMA routes each head into O^T layout.
"""

import numpy as np

P = 128
B, S, D = 2, 2048, 1024
H, DK = 16, 64
COLS = 256          # qkv columns per core (4 heads)
KC = D // P         # 8 contraction chunks for the projections
TT = 512            # token block (matmul free dim)
NJ = S // TT        # 4 token blocks
NT = S // P         # 16 token tiles
NKT = S // P        # 16 key tiles
VW = 65             # per-head AV lhsT width: ones column + 64 v-dims

_CACHE = {}


def _build():
    import concourse.bass as bass
    import concourse.tile as tile
    from concourse import bacc, mybir

    f32 = mybir.dt.float32
    f32r = mybir.dt.float32r
    Exp = mybir.ActivationFunctionType.Exp

    bf16 = mybir.dt.bfloat16
    nc = bacc.Bacc(
        "TRN2", target_bir_lowering=False, debug=False,
        enable_asserts=False, num_devices=8,
    )
    xh_d = nc.dram_tensor("xh", [S, D], bf16, kind="ExternalInput").ap()
    xl_d = nc.dram_tensor("xl", [S, D], bf16, kind="ExternalInput").ap()
    wq_d = nc.dram_tensor("wq", [D, COLS], f32, kind="ExternalInput").ap()
    wk_d = nc.dram_tensor("wk", [D, COLS], f32, kind="ExternalInput").ap()
    wv_d = nc.dram_tensor("wv", [D, COLS], f32, kind="ExternalInput").ap()
    wo_d = nc.dram_tensor("wo", [COLS, D], f32, kind="ExternalInput").ap()
    bq_d = nc.dram_tensor("bq", [COLS], f32, kind="ExternalInput").ap()
    bk_d = nc.dram_tensor("bk", [COLS], f32, kind="ExternalInput").ap()
    out_d = nc.dram_tensor("out_t", [D, S], f32, kind="ExternalOutput").ap()

    with tile.TileContext(nc) as tc:
        with (
            tc.tile_pool(name="const", bufs=1) as const,
            tc.tile_pool(name="wst", bufs=1) as wst,
            tc.tile_pool(name="wpool", bufs=1) as wpool,
            tc.tile_pool(name="persist", bufs=1) as persist,
            tc.tile_pool(name="xhl", bufs=1) as xhl,
            tc.tile_pool(name="xtp", bufs=2) as xtp,
            tc.tile_pool(name="exps", bufs=3) as exps,
            tc.tile_pool(name="stage", bufs=3) as stage,
            tc.tile_pool(name="outst", bufs=4) as outst,
            tc.tile_pool(name="ps_sc", bufs=1, space="PSUM") as ps_sc,
            tc.tile_pool(name="ps_acc", bufs=2, space="PSUM") as ps_acc,
            tc.tile_pool(name="ps_u", bufs=2, space="PSUM") as ps_u,
        ):
            ones32 = const.tile([P, VW], f32, tag="ones32")
            nc.vector.memset(ones32[:], 1.0)
            ones_r = const.tile([P, VW], f32r, tag="ones_r")
            nc.vector.tensor_copy(ones_r[:], ones32[:])

            # ---- weights: DMA fp32 -> convert to f32r on DVE ----
            def load_w(dram, shape_free, name):
                st = wst.tile([P, KC, shape_free], f32, tag="wstage", name="wstage")
                nc.sync.dma_start(st[:], dram.rearrange("(o p) f -> p o f", p=P))
                wr = wpool.tile([P, KC, shape_free], f32r, tag=f"w_{name}",
                                name=f"w_{name}")
                nc.vector.tensor_copy(wr[:], st[:])
                return wr

            wq_r = load_w(wq_d, COLS, "q")
            wk_r = load_w(wk_d, COLS, "k")
            wv_r = load_w(wv_d, COLS, "v")
            wo_st = wst.tile([P, 2, D], f32, tag="wstage", name="wostage")
            nc.sync.dma_start(wo_st[:], wo_d.rearrange("(o p) f -> p o f", p=P))
            wo_r = wpool.tile([P, 2, D], f32r, tag="w_o")
            nc.vector.tensor_copy(wo_r[:], wo_st[:])

            bq_sb = const.tile([P, 2], f32, tag="bq")
            nc.sync.dma_start(bq_sb[:], bq_d.rearrange("(o p) -> p o", p=P))
            bk_sb = const.tile([P, 2], f32, tag="bk")
            nc.sync.dma_start(bk_sb[:], bk_d.rearrange("(o p) -> p o", p=P))

            # persistent activations
            qT = persist.tile([P, 2, S], f32r, tag="qT")    # [qcol, tok]
            kT = persist.tile([P, 2, S], f32r, tag="kT")    # [kcol, tok]
            vt = persist.tile([P, NT, 4 * VW], f32r, tag="vt")  # [tok, h*(1|V)]
            oT = persist.tile([P, 2, S], f32r, tag="oT")    # [vdim, tok]

            # ones column (index 64 of each head's VW slice)
            vt_heads = vt[:].rearrange("p t (h c) -> p t h c", c=VW)
            nc.vector.tensor_copy(
                vt_heads[:, :, :, 64],
                ones32[:, :NT * 4].rearrange("p (t h) -> p t h", h=4),
            )

            # ---- phase 0/1: x transpose + QKV projections, per token block ----
            for j in range(NJ):
                xT = xtp.tile([P, KC, TT], f32r, tag="xT")
                xth = xhl.tile([P, KC, TT], bf16, tag="xth", name="xth")
                nc.sync.dma_start_transpose(xth[:], xh_d[bass.ts(j, TT), :])
                xtl = xhl.tile([P, KC, TT], bf16, tag="xtl", name="xtl")
                nc.sync.dma_start_transpose(xtl[:], xl_d[bass.ts(j, TT), :])
                nc.vector.tensor_tensor(
                    xT[:], xth[:], xtl[:], mybir.AluOpType.add
                )

                # Q^T, K^T: [qcol, tok] with bias
                for (wmat, bsb, dstT) in ((wq_r, bq_sb, qT), (wk_r, bk_sb, kT)):
                    for ct in range(2):
                        acc = ps_u.tile([P, TT], f32, tag="u", name="qk_acc")
                        for kc in range(KC):
                            nc.tensor.matmul(
                                acc[:], wmat[:, kc, bass.ts(ct, P)], xT[:, kc, :],
                                start=(kc == 0), stop=(kc == KC - 1),
                            )
                        nc.vector.tensor_scalar_add(
                            dstT[:, ct, bass.ts(j, TT)], acc[:], bsb[:, ct : ct + 1]
                        )

                # V: [tok, vcol]
                for ts in range(TT // P):
                    acc = ps_u.tile([P, COLS], f32, tag="u", name="v_acc")
                    for kc in range(KC):
                        nc.tensor.matmul(
                            acc[:], xT[:, kc, bass.ts(ts, P)], wv_r[:, kc, :],
                            start=(kc == 0), stop=(kc == KC - 1),
                        )
                    tt = 4 * j + ts
                    nc.vector.tensor_copy(
                        vt_heads[:, tt, :, 0:64],
                        acc[:].rearrange("p (h c) -> p h c", c=64),
                    )

            # shared scores PSUM tensor: 4 slots x [128, 512] = 4 banks
            big_sc = ps_sc.tile([P, 4, TT], f32, tag="sc")

            # ---- phase 2 + 3 interleaved over token blocks ----
            for j in range(NJ):
                for p in range(2):
                    o_ps = [
                        ps_acc.tile([VW, TT], f32, tag="acc", name=f"o_ps{i}")
                        for i in range(2)
                    ]
                    # software-pipelined emission: scores run 2 k-chunks ahead,
                    # AV trails exp by one, so PE always has ready work while
                    # ACT's ~1.1us exp latency is in flight.
                    def sc_emit(kc):
                        base = (2 * kc) % 4
                        for i in range(2):
                            lo, hi = 64 * i, 64 * i + 64
                            nc.tensor.matmul(
                                big_sc[:, base + i, :],
                                kT[lo:hi, p, bass.ts(kc, P)],
                                qT[lo:hi, p, bass.ts(j, TT)],
                                start=True, stop=True,
                            )

                    def av_emit(kc, ex):
                        for i in range(2):
                            h = 2 * p + i
                            nc.tensor.matmul(
                                o_ps[i][:],
                                vt[:, kc, bass.ds(VW * h, VW)],
                                ex[:, i, :],
                                start=(kc == 0), stop=(kc == NKT - 1),
                            )

                    sc_emit(0)
                    sc_emit(1)
                    prev = None
                    for kc in range(NKT):
                        base = (2 * kc) % 4
                        ex = exps.tile([P, 2, TT], f32r, tag="exp", name="ex")
                        nc.scalar.activation(
                            ex[:], big_sc[:, base : base + 2, :], Exp,
                            scale=0.125,
                        )
                        if prev is not None:
                            av_emit(kc - 1, prev)
                        if kc + 2 < NKT:
                            sc_emit(kc + 2)
                        prev = ex
                    av_emit(NKT - 1, prev)

                    # normalize both heads into O^T via recip/broadcast/mult/DMA
                    o32 = stage.tile([P, TT], f32r, tag="o32", name="o32")
                    for i in range(2):
                        # free o_ps quickly: one copy to SBUF, then normalize
                        osb = stage.tile([P, TT], f32r, tag="osb", name="osb")
                        nc.vector.tensor_copy(osb[0:VW, :], o_ps[i][:])
                        # broadcast the sums row via PE ones outer-product
                        rbc = ps_u.tile([64, TT], f32, tag="u", name="rbc")
                        nc.tensor.matmul(
                            rbc[:], ones_r[64:65, 0:64], osb[64:65, :],
                            start=True, stop=True,
                        )
                        rbs = stage.tile([64, TT], f32, tag="rbs", name="rbs")
                        nc.vector.reciprocal_approx_fast(rbs[:], rbc[:])
                        onrm = stage.tile([P, TT], f32r, tag="onrm", name="onrm")
                        nc.vector.tensor_tensor(
                            onrm[0:64, :], osb[0:64, :], rbs[:],
                            mybir.AluOpType.mult,
                        )
                        nc.sync.dma_start(
                            o32[bass.ds(64 * i, 64), :], onrm[0:64, :]
                        )
                    nc.vector.tensor_copy(oT[:, p, bass.ts(j, TT)], o32[:])

                # partial output projection for this token block
                for oc in range(D // P):
                    acc = ps_u.tile([P, TT], f32, tag="u", name="wo_acc")
                    for vc in range(2):
                        nc.tensor.matmul(
                            acc[:], wo_r[:, vc, bass.ts(oc, P)],
                            oT[:, vc, bass.ts(j, TT)],
                            start=(vc == 0), stop=(vc == 1),
                        )
                    st = outst.tile([P, TT], f32, tag="outst", name="outst")
                    nc.vector.tensor_copy(st[:], acc[:])
                    nc.sync.dma_start(out_d[bass.ts(oc, P), bass.ts(j, TT)], st[:])

    nc.compile()
    return nc


def make_in_maps(x, Wq, bq, Wk, bk, Wv, Wo):
    import ml_dtypes

    xh = [None, None]
    xl = [None, None]
    for b in range(B):
        hi = x[b].astype(ml_dtypes.bfloat16)
        lo = (x[b] - hi.astype(np.float32)).astype(ml_dtypes.bfloat16)
        xh[b], xl[b] = np.ascontiguousarray(hi), np.ascontiguousarray(lo)

    in_maps = []
    for c in range(8):
        b, g = divmod(c, 4)
        cs = slice(COLS * g, COLS * (g + 1))
        in_maps.append({
            "xh": xh[b],
            "xl": xl[b],
            "wq": np.ascontiguousarray(Wq[:, cs]),
            "wk": np.ascontiguousarray(Wk[:, cs]),
            "wv": np.ascontiguousarray(Wv[:, cs]),
            "wo": np.ascontiguousarray(Wo[cs, :]),
            "bq": np.ascontiguousarray(bq[cs]),
            "bk": np.ascontiguousarray(bk[cs]),
        })
    return in_maps


def kernel(x, Wq, bq, Wk, bk, Wv, bv, Wo, bo):
    from concourse import bass_utils

    x = np.asarray(x, dtype=np.float32)
    Wq = np.asarray(Wq, dtype=np.float32)
    Wk = np.asarray(Wk, dtype=np.float32)
    Wv = np.asarray(Wv, dtype=np.float32)
    Wo = np.asarray(Wo, dtype=np.float32)
    bq = np.asarray(bq, dtype=np.float32)
    bk = np.asarray(bk, dtype=np.float32)
    bv = np.asarray(bv, dtype=np.float32)
    bo = np.asarray(bo, dtype=np.float32)

    if "nc" not in _CACHE:
        _CACHE["nc"] = _build()
    nc = _CACHE["nc"]

    in_maps = make_in_maps(x, Wq, bq, Wk, bk, Wv, Wo)
    res = bass_utils.run_bass_kernel_spmd(nc, in_maps, core_ids=list(range(8)))

    out = np.zeros((B, S, D), dtype=np.float32)
    for c in range(8):
        out[c // 4] += res.results[c]["out_t"].T
    out += bo + bv @ Wo
    return out



# revision 5
# speedup vs baseline: 1.2017x; 1.2017x over previous
"""Multi-head attention (B=2, S=2048, D=1024, H=16, dk=64) on 8 Trainium2
NeuronCores via Bass/Tile.

Sharding: core c handles batch b = c//4 and head-group g = c%4 (4 heads,
256 qkv columns).  Each core computes its QKV projection slices, 4 heads of
attention, and a partial output projection against its 256-row slice of Wo.
The host sums the 4 partial outputs per batch (row-sharded Wo => partial
sums) and folds in the biases bo and bv@Wo (softmax rows sum to 1, so the
V-bias contributes exactly bv@Wo per token).

v3 design notes (vs v2's f32r):
- Everything bf16: fp32 feeds the PE at half the bf16 streaming rate
  (measured 1.2 rows/ns vs 2.4), so bf16 operands double matmul throughput.
  PSUM accumulation stays f32.  Measured numeric impact ~2.3e-3 rel_l2.
- Weights are cast to bf16 host-side -> straight DMA, no on-chip casts.
- x is DMA-transposed once (bf16, no hi/lo split).
- The exp on the ACT engine (~1.1us per [128,2,512] chunk) is the critical
  path of the attention phase.  Q-projections for blocks 1-3 and all output
  projections are emitted as "filler" PE work inside the attention kc loop,
  occupying PE stall slots under the ACT-bound pipeline instead of
  serializing before/after it.
- Normalization per head: PSUM->SBUF copy (bf16), PE ones-outer-product to
  broadcast the denominator row, DVE fast reciprocal, DVE multiply.  Head 0
  writes O^T directly; head 1 goes through one partition-shifting
  SBUF->SBUF DMA.
"""

import numpy as np

P = 128
B, S, D = 2, 2048, 1024
H, DK = 16, 64
COLS = 256          # qkv columns per core (4 heads)
KC = D // P         # 8 contraction chunks for the projections
TT = 512            # token block (matmul free dim)
NJ = S // TT        # 4 token blocks
NT = S // P         # 16 token tiles
NKT = S // P        # 16 key tiles
VW = 65             # per-head AV lhsT width: 64 v-dims + ones column

_CACHE = {}


def _build():
    import concourse.bass as bass
    import concourse.tile as tile
    from concourse import bacc, mybir

    f32 = mybir.dt.float32
    bf16 = mybir.dt.bfloat16
    Exp = mybir.ActivationFunctionType.Exp

    nc = bacc.Bacc(
        "TRN2", target_bir_lowering=False, debug=False,
        enable_asserts=False, num_devices=8,
    )
    xh_d = nc.dram_tensor("xh", [S, D], bf16, kind="ExternalInput").ap()
    wq_d = nc.dram_tensor("wq", [D, COLS], bf16, kind="ExternalInput").ap()
    wk_d = nc.dram_tensor("wk", [D, COLS], bf16, kind="ExternalInput").ap()
    wv_d = nc.dram_tensor("wv", [D, COLS], bf16, kind="ExternalInput").ap()
    wo_d = nc.dram_tensor("wo", [COLS, D], bf16, kind="ExternalInput").ap()
    bq_d = nc.dram_tensor("bq", [COLS], f32, kind="ExternalInput").ap()
    bk_d = nc.dram_tensor("bk", [COLS], f32, kind="ExternalInput").ap()
    out_d = nc.dram_tensor("out_t", [D, S], f32, kind="ExternalOutput").ap()

    with tile.TileContext(nc) as tc:
        with (
            tc.tile_pool(name="const", bufs=1) as const,
            tc.tile_pool(name="wpool", bufs=1) as wpool,
            tc.tile_pool(name="persist", bufs=1) as persist,
            tc.tile_pool(name="exps", bufs=3) as exps,
            tc.tile_pool(name="stage", bufs=4) as stage,
            tc.tile_pool(name="outst", bufs=4) as outst,
            tc.tile_pool(name="ps_sc", bufs=1, space="PSUM") as ps_sc,
            tc.tile_pool(name="ps_acc", bufs=1, space="PSUM") as ps_acc,
            tc.tile_pool(name="ps_u", bufs=2, space="PSUM") as ps_u,
        ):
            ones_b = const.tile([P, VW], bf16, tag="ones_b")
            nc.vector.memset(ones_b[:], 1.0)

            # ---- weights: already bf16 in DRAM (host-side cast) ----
            wq_r = wpool.tile([P, KC, COLS], bf16, tag="w_q")
            nc.sync.dma_start(wq_r[:], wq_d.rearrange("(o p) f -> p o f", p=P))
            wk_r = wpool.tile([P, KC, COLS], bf16, tag="w_k")
            nc.sync.dma_start(wk_r[:], wk_d.rearrange("(o p) f -> p o f", p=P))
            wv_r = wpool.tile([P, KC, COLS], bf16, tag="w_v")
            nc.sync.dma_start(wv_r[:], wv_d.rearrange("(o p) f -> p o f", p=P))
            wo_r = wpool.tile([P, 2, D], bf16, tag="w_o")
            nc.sync.dma_start(wo_r[:], wo_d.rearrange("(o p) f -> p o f", p=P))

            bq_sb = const.tile([P, 2], f32, tag="bq")
            nc.sync.dma_start(bq_sb[:], bq_d.rearrange("(o p) -> p o", p=P))
            bk_sb = const.tile([P, 2], f32, tag="bk")
            nc.sync.dma_start(bk_sb[:], bk_d.rearrange("(o p) -> p o", p=P))

            # persistent activations (all bf16)
            qT = persist.tile([P, 2, S], bf16, tag="qT")    # [qcol, tok]
            kT = persist.tile([P, 2, S], bf16, tag="kT")    # [kcol, tok]
            vt = persist.tile([P, NT, 4 * VW], bf16, tag="vt")  # [tok, h*(V|1)]
            oT = persist.tile([P, 2, S], bf16, tag="oT")    # [vdim, tok]
            xTs = [persist.tile([P, KC, TT], bf16, tag=f"xT{j}", name=f"xT{j}")
                   for j in range(NJ)]

            # ones column (index 64 of each head's VW slice)
            vt_heads = vt[:].rearrange("p t (h c) -> p t h c", c=VW)
            nc.vector.tensor_copy(
                vt_heads[:, :, :, 64],
                ones_b[:, :NT * 4].rearrange("p (t h) -> p t h", h=4),
            )

            for j in range(NJ):
                nc.sync.dma_start_transpose(xTs[j][:], xh_d[bass.ts(j, TT), :])

            # ---- projection emitters ----
            def qk_proj_ct(j, wmat, bsb, dstT, ct, acc, kc0, kc1):
                for kc in range(kc0, kc1):
                    nc.tensor.matmul(
                        acc[:], wmat[:, kc, bass.ts(ct, P)], xTs[j][:, kc, :],
                        start=(kc == 0), stop=(kc == KC - 1),
                    )
                if kc1 == KC:
                    nc.vector.tensor_scalar_add(
                        dstT[:, ct, bass.ts(j, TT)], acc[:], bsb[:, ct : ct + 1]
                    )

            def v_proj(j, ts_):
                acc = ps_u.tile([P, COLS], f32, tag="u", name="v_acc")
                for kc in range(KC):
                    nc.tensor.matmul(
                        acc[:], xTs[j][:, kc, bass.ts(ts_, P)], wv_r[:, kc, :],
                        start=(kc == 0), stop=(kc == KC - 1),
                    )
                tt = 4 * j + ts_
                nc.vector.tensor_copy(
                    vt_heads[:, tt, :, 0:64],
                    acc[:].rearrange("p (h c) -> p h c", c=64),
                )

            # ---- phase A: all K and V projections + Q(0) up front ----
            for j in range(NJ):
                for ct in range(2):
                    acc = ps_u.tile([P, TT], f32, tag="u", name="k_acc")
                    qk_proj_ct(j, wk_r, bk_sb, kT, ct, acc, 0, KC)
                for ts_ in range(TT // P):
                    v_proj(j, ts_)
            for ct in range(2):
                acc = ps_u.tile([P, TT], f32, tag="u", name="q_acc")
                qk_proj_ct(0, wq_r, bq_sb, qT, ct, acc, 0, KC)

            # ---- filler queue: atomic groups of PE work (Q(1..3) column
            # tiles, out-projection units) drained one step per kc slot
            # inside the ACT-bound attention loop.  A multi-slot group is
            # only started when it fits in the current block's remaining
            # slots, so a ps_u accumulation never straddles the block
            # boundary where the norm's rbc tiles rotate through ps_u
            # (that interleaving could deadlock the in-order PE queue). ----
            fillers = []   # list of groups; group = list of step closures
            active = []    # remaining steps of the currently started group

            def qproj_group(j, ct):
                box = {}
                def step(kc0, box=box):
                    if kc0 == 0:
                        box["acc"] = ps_u.tile([P, TT], f32, tag="u",
                                               name="q_acc")
                    qk_proj_ct(j, wq_r, bq_sb, qT, ct, box["acc"],
                               kc0, kc0 + 2)
                return [lambda kc0=kc0: step(kc0) for kc0 in range(0, KC, 2)]

            def outproj_group(j, oc):
                def step():
                    acc = ps_u.tile([P, TT], f32, tag="u", name="wo_acc")
                    for vc in range(2):
                        nc.tensor.matmul(
                            acc[:], wo_r[:, vc, bass.ts(oc, P)],
                            oT[:, vc, bass.ts(j, TT)],
                            start=(vc == 0), stop=(vc == 1),
                        )
                    st = outst.tile([P, TT], f32, tag="outst", name="outst")
                    nc.vector.tensor_copy(st[:], acc[:])
                    nc.sync.dma_start(
                        out_d[bass.ts(oc, P), bass.ts(j, TT)], st[:]
                    )
                return [step]

            def drain_filler(slots_left):
                if not active:
                    for gi, grp in enumerate(fillers):
                        if len(grp) <= slots_left:
                            active.extend(fillers.pop(gi))
                            break
                    else:
                        return
                active.pop(0)()

            for j in range(1, NJ):
                for ct in range(2):
                    fillers.append(qproj_group(j, ct))

            # ---- normalization, split in two parts: the PSUM->SBUF copies
            # (the only o_ps reads) are emitted right after the last AV so
            # the next block's o_ps alloc records them; the arithmetic runs
            # after the next block's first scores so PE/ACT keep flowing ----
            def norm_copies(o_ps):
                osbs = []
                for i in range(2):
                    osb = stage.tile([VW, TT], bf16, tag="osb", name="osb")
                    nc.vector.tensor_copy(osb[:], o_ps[0:VW, i, :])
                    osbs.append(osb)
                return osbs

            def norm_arith(j, p, osbs):
                for i in range(2):
                    osb = osbs[i]
                    rbc = ps_u.tile([64, TT], f32, tag="u", name="rbc")
                    nc.tensor.matmul(
                        rbc[:], ones_b[64:65, 0:64], osb[64:65, :],
                        start=True, stop=True,
                    )
                    rbs = stage.tile([64, TT], f32, tag="rbs", name="rbs")
                    nc.vector.reciprocal_approx_fast(rbs[:], rbc[:])
                    if i == 0:
                        nc.vector.tensor_tensor(
                            oT[0:64, p, bass.ts(j, TT)], osb[0:64, :], rbs[:],
                            mybir.AluOpType.mult,
                        )
                    else:
                        onrm = stage.tile([64, TT], bf16, tag="onrm",
                                          name="onrm")
                        nc.vector.tensor_tensor(
                            onrm[:], osb[0:64, :], rbs[:], mybir.AluOpType.mult
                        )
                        nc.sync.dma_start(
                            oT[64:128, p, bass.ts(j, TT)], onrm[:]
                        )

            # shared scores PSUM tensor: 4 slots x [128, 512] = 4 banks
            big_sc = ps_sc.tile([P, 4, TT], f32, tag="sc")

            # ---- attention: ACT-paced kc pipeline with PE fillers ----
            pending_norm = None
            for j in range(NJ):
                for p in range(2):
                    o_ps = ps_acc.tile([P, 2, TT], f32, tag="acc", name="o_ps")

                    def sc_emit(kc, j=j, p=p):
                        base = (2 * kc) % 4
                        for i in range(2):
                            lo, hi = 64 * i, 64 * i + 64
                            nc.tensor.matmul(
                                big_sc[:, base + i, :],
                                kT[lo:hi, p, bass.ts(kc, P)],
                                qT[lo:hi, p, bass.ts(j, TT)],
                                start=True, stop=True,
                            )

                    def av_emit(kc, ex, p=p, o_ps=o_ps):
                        for i in range(2):
                            h = 2 * p + i
                            nc.tensor.matmul(
                                o_ps[0:VW, i, :],
                                vt[:, kc, bass.ds(VW * h, VW)],
                                ex[:, i, :],
                                start=(kc == 0), stop=(kc == NKT - 1),
                            )

                    sc_emit(0)
                    sc_emit(1)
                    if pending_norm is not None:
                        pending_norm()
                        pending_norm = None
                    prev = None
                    for kc in range(NKT):
                        base = (2 * kc) % 4
                        ex = exps.tile([P, 2, TT], bf16, tag="exp", name="ex")
                        nc.scalar.activation(
                            ex[:], big_sc[:, base : base + 2, :], Exp,
                            scale=0.125,
                        )
                        if prev is not None:
                            av_emit(kc - 1, prev)
                        if kc + 2 < NKT:
                            sc_emit(kc + 2)
                        drain_filler(NKT - kc)
                        prev = ex
                    av_emit(NKT - 1, prev)
                    osbs = norm_copies(o_ps)
                    pending_norm = (
                        lambda j=j, p=p, osbs=osbs: norm_arith(j, p, osbs)
                    )
                for oc in range(D // P):
                    fillers.append(outproj_group(j, oc))
            pending_norm()
            while fillers or active:
                drain_filler(NKT)

    nc.compile()
    return nc


def make_in_maps(x, Wq, bq, Wk, bk, Wv, Wo):
    import ml_dtypes

    bf = ml_dtypes.bfloat16
    xh = [np.ascontiguousarray(x[b].astype(bf)) for b in range(B)]

    in_maps = []
    for c in range(8):
        b, g = divmod(c, 4)
        cs = slice(COLS * g, COLS * (g + 1))
        in_maps.append({
            "xh": xh[b],
            "wq": np.ascontiguousarray(Wq[:, cs].astype(bf)),
            "wk": np.ascontiguousarray(Wk[:, cs].astype(bf)),
            "wv": np.ascontiguousarray(Wv[:, cs].astype(bf)),
            "wo": np.ascontiguousarray(Wo[cs, :].astype(bf)),
            "bq": np.ascontiguousarray(bq[cs]),
            "bk": np.ascontiguousarray(bk[cs]),
        })
    return in_maps


def kernel(x, Wq, bq, Wk, bk, Wv, bv, Wo, bo):
    from concourse import bass_utils

    x = np.asarray(x, dtype=np.float32)
    Wq = np.asarray(Wq, dtype=np.float32)
    Wk = np.asarray(Wk, dtype=np.float32)
    Wv = np.asarray(Wv, dtype=np.float32)
    Wo = np.asarray(Wo, dtype=np.float32)
    bq = np.asarray(bq, dtype=np.float32)
    bk = np.asarray(bk, dtype=np.float32)
    bv = np.asarray(bv, dtype=np.float32)
    bo = np.asarray(bo, dtype=np.float32)

    if "nc" not in _CACHE:
        _CACHE["nc"] = _build()
    nc = _CACHE["nc"]

    in_maps = make_in_maps(x, Wq, bq, Wk, bk, Wv, Wo)
    res = bass_utils.run_bass_kernel_spmd(nc, in_maps, core_ids=list(range(8)))

    out = np.zeros((B, S, D), dtype=np.float32)
    for c in range(8):
        out[c // 4] += res.results[c]["out_t"].T
    out += bo + bv @ Wo
    return out


# revision 8
# speedup vs baseline: 1.6250x; 1.3522x over previous
"""Multi-head attention (B=2, S=2048, D=1024, H=16, dk=64) on 8 Trainium2
NeuronCores via Bass/Tile.

Sharding: core c handles batch b = c//4 and head-group g = c%4 (4 heads,
256 qkv columns).  Each core computes its QKV projection slices, 4 heads of
attention, and a partial output projection against its 256-row slice of Wo.
The host sums the 4 partial outputs per batch (row-sharded Wo => partial
sums) and folds in the biases bo and bv@Wo (softmax rows sum to 1, so the
V-bias contributes exactly bv@Wo per token).

v3 design notes (vs v2's f32r):
- Everything bf16: fp32 feeds the PE at half the bf16 streaming rate
  (measured 1.2 rows/ns vs 2.4), so bf16 operands double matmul throughput.
  PSUM accumulation stays f32.  Measured numeric impact ~2.3e-3 rel_l2.
- Weights are cast to bf16 host-side -> straight DMA, no on-chip casts.
- x is DMA-transposed once (bf16, no hi/lo split).
- The exp on the ACT engine (~1.1us per [128,2,512] chunk) is the critical
  path of the attention phase.  Q-projections for blocks 1-3 and all output
  projections are emitted as "filler" PE work inside the attention kc loop,
  occupying PE stall slots under the ACT-bound pipeline instead of
  serializing before/after it.
- Normalization per head: PSUM->SBUF copy (bf16), PE ones-outer-product to
  broadcast the denominator row, DVE fast reciprocal, DVE multiply.  Head 0
  writes O^T directly; head 1 goes through one partition-shifting
  SBUF->SBUF DMA.
"""

import numpy as np

P = 128
B, S, D = 2, 2048, 1024
H, DK = 16, 64
COLS = 256          # qkv columns per core (4 heads)
KC = D // P         # 8 contraction chunks for the projections
TT = 512            # token block (matmul free dim)
NJ = S // TT        # 4 token blocks
NT = S // P         # 16 token tiles
NKT = S // P        # 16 key tiles
VW = 65             # per-head AV lhsT width: 64 v-dims + ones column

_CACHE = {}


def _build():
    import concourse.bass as bass
    import concourse.tile as tile
    from concourse import bacc, mybir

    f32 = mybir.dt.float32
    bf16 = mybir.dt.bfloat16
    Exp = mybir.ActivationFunctionType.Exp

    nc = bacc.Bacc(
        "TRN2", target_bir_lowering=False, debug=False,
        enable_asserts=False, num_devices=8,
    )
    xh_d = nc.dram_tensor("xh", [S, D], bf16, kind="ExternalInput").ap()
    wq_d = nc.dram_tensor("wq", [D, COLS], bf16, kind="ExternalInput").ap()
    wk_d = nc.dram_tensor("wk", [D, COLS], bf16, kind="ExternalInput").ap()
    wv_d = nc.dram_tensor("wv", [D, COLS], bf16, kind="ExternalInput").ap()
    wo_d = nc.dram_tensor("wo", [COLS, D], bf16, kind="ExternalInput").ap()
    bq_d = nc.dram_tensor("bq", [COLS], f32, kind="ExternalInput").ap()
    bk_d = nc.dram_tensor("bk", [COLS], f32, kind="ExternalInput").ap()
    out_d = nc.dram_tensor("out_t", [D, S], f32, kind="ExternalOutput").ap()

    with tile.TileContext(nc) as tc:
        with (
            tc.tile_pool(name="const", bufs=1) as const,
            tc.tile_pool(name="wpool", bufs=1) as wpool,
            tc.tile_pool(name="persist", bufs=1) as persist,
            tc.tile_pool(name="exps", bufs=3) as exps,
            tc.tile_pool(name="stage", bufs=4) as stage,
            tc.tile_pool(name="outst", bufs=4) as outst,
            tc.tile_pool(name="ps_sc", bufs=1, space="PSUM") as ps_sc,
            tc.tile_pool(name="ps_acc", bufs=1, space="PSUM") as ps_acc,
            tc.tile_pool(name="ps_u", bufs=2, space="PSUM") as ps_u,
        ):
            ones_b = const.tile([P, VW], bf16, tag="ones_b")
            nc.vector.memset(ones_b[:], 1.0)

            # ---- weights: already bf16 in DRAM (host-side cast) ----
            wq_r = wpool.tile([P, KC, COLS], bf16, tag="w_q")
            nc.sync.dma_start(wq_r[:], wq_d.rearrange("(o p) f -> p o f", p=P))
            wk_r = wpool.tile([P, KC, COLS], bf16, tag="w_k")
            nc.sync.dma_start(wk_r[:], wk_d.rearrange("(o p) f -> p o f", p=P))
            wv_r = wpool.tile([P, KC, COLS], bf16, tag="w_v")
            nc.sync.dma_start(wv_r[:], wv_d.rearrange("(o p) f -> p o f", p=P))
            wo_r = wpool.tile([P, 2, D], bf16, tag="w_o")
            nc.sync.dma_start(wo_r[:], wo_d.rearrange("(o p) f -> p o f", p=P))

            bq_sb = const.tile([P, 2], f32, tag="bq")
            nc.sync.dma_start(bq_sb[:], bq_d.rearrange("(o p) -> p o", p=P))
            bk_sb = const.tile([P, 2], f32, tag="bk")
            nc.sync.dma_start(bk_sb[:], bk_d.rearrange("(o p) -> p o", p=P))

            # persistent activations (all bf16)
            qT = persist.tile([P, 2, S], bf16, tag="qT")    # [qcol, tok]
            kT = persist.tile([P, 2, S], bf16, tag="kT")    # [kcol, tok]
            vt = persist.tile([P, NT, 4 * VW], bf16, tag="vt")  # [tok, h*(V|1)]
            oT = persist.tile([P, 2, S], bf16, tag="oT")    # [vdim, tok]
            xTs = [persist.tile([P, KC, TT], bf16, tag=f"xT{j}", name=f"xT{j}")
                   for j in range(NJ)]

            # ones column (index 64 of each head's VW slice)
            vt_heads = vt[:].rearrange("p t (h c) -> p t h c", c=VW)
            nc.vector.tensor_copy(
                vt_heads[:, :, :, 64],
                ones_b[:, :NT * 4].rearrange("p (t h) -> p t h", h=4),
            )

            for j in range(NJ):
                nc.sync.dma_start_transpose(xTs[j][:], xh_d[bass.ts(j, TT), :])

            # ---- projection emitters ----
            def qk_proj_ct(j, wmat, bsb, dstT, ct, acc, kc0, kc1):
                for kc in range(kc0, kc1):
                    nc.tensor.matmul(
                        acc[:], wmat[:, kc, bass.ts(ct, P)], xTs[j][:, kc, :],
                        start=(kc == 0), stop=(kc == KC - 1),
                    )
                if kc1 == KC:
                    nc.vector.tensor_scalar_add(
                        dstT[:, ct, bass.ts(j, TT)], acc[:], bsb[:, ct : ct + 1]
                    )

            def v_proj(j, ts_):
                acc = ps_u.tile([P, COLS], f32, tag="u", name="v_acc")
                for kc in range(KC):
                    nc.tensor.matmul(
                        acc[:], xTs[j][:, kc, bass.ts(ts_, P)], wv_r[:, kc, :],
                        start=(kc == 0), stop=(kc == KC - 1),
                    )
                tt = 4 * j + ts_
                nc.vector.tensor_copy(
                    vt_heads[:, tt, :, 0:64],
                    acc[:].rearrange("p (h c) -> p h c", c=64),
                )

            # ---- phase A: all K and V projections + Q(0) up front ----
            for j in range(NJ):
                for ct in range(2):
                    acc = ps_u.tile([P, TT], f32, tag="u", name="k_acc")
                    qk_proj_ct(j, wk_r, bk_sb, kT, ct, acc, 0, KC)
                for ts_ in range(TT // P):
                    v_proj(j, ts_)
            for ct in range(2):
                acc = ps_u.tile([P, TT], f32, tag="u", name="q_acc")
                qk_proj_ct(0, wq_r, bq_sb, qT, ct, acc, 0, KC)

            # ---- filler queue: atomic groups of PE work (Q(1..3) column
            # tiles, out-projection units) drained one step per kc slot
            # inside the ACT-bound attention loop.  A multi-slot group is
            # only started when it fits in the current block's remaining
            # slots, so a ps_u accumulation never straddles the block
            # boundary where the norm's rbc tiles rotate through ps_u
            # (that interleaving could deadlock the in-order PE queue). ----
            fillers = []   # list of groups; group = list of step closures
            active = []    # remaining steps of the currently started group

            def qproj_group(j, ct):
                box = {}
                def step(kc0, box=box):
                    if kc0 == 0:
                        box["acc"] = ps_u.tile([P, TT], f32, tag="u",
                                               name="q_acc")
                    qk_proj_ct(j, wq_r, bq_sb, qT, ct, box["acc"],
                               kc0, kc0 + 2)
                return [lambda kc0=kc0: step(kc0) for kc0 in range(0, KC, 2)]

            def outproj_group(j, oc):
                def step():
                    acc = ps_u.tile([P, TT], f32, tag="u", name="wo_acc")
                    for vc in range(2):
                        nc.tensor.matmul(
                            acc[:], wo_r[:, vc, bass.ts(oc, P)],
                            oT[:, vc, bass.ts(j, TT)],
                            start=(vc == 0), stop=(vc == 1),
                        )
                    st = outst.tile([P, TT], f32, tag="outst", name="outst")
                    nc.vector.tensor_copy(st[:], acc[:])
                    nc.sync.dma_start(
                        out_d[bass.ts(oc, P), bass.ts(j, TT)], st[:]
                    )
                return [step]

            def drain_filler(slots_left):
                if not active:
                    for gi, grp in enumerate(fillers):
                        if len(grp) <= slots_left:
                            active.extend(fillers.pop(gi))
                            break
                    else:
                        return
                active.pop(0)()

            for j in range(1, NJ):
                for ct in range(2):
                    fillers.append(qproj_group(j, ct))

            # ---- normalization, split in two parts: the PSUM->SBUF copies
            # (the only o_ps reads) are emitted right after the last AV so
            # the next block's o_ps alloc records them; the arithmetic runs
            # after the next block's first scores so PE/ACT keep flowing ----
            def norm_copies(o_ps):
                osbs = []
                for i in range(2):
                    osb = stage.tile([VW, TT], bf16, tag="osb", name="osb")
                    nc.vector.tensor_copy(osb[:], o_ps[0:VW, i, :])
                    osbs.append(osb)
                return osbs

            def norm_arith(j, p, osbs):
                for i in range(2):
                    osb = osbs[i]
                    rbc = ps_u.tile([64, TT], f32, tag="u", name="rbc")
                    nc.tensor.matmul(
                        rbc[:], ones_b[64:65, 0:64], osb[64:65, :],
                        start=True, stop=True,
                    )
                    rbs = stage.tile([64, TT], f32, tag="rbs", name="rbs")
                    nc.vector.reciprocal_approx_fast(rbs[:], rbc[:])
                    if i == 0:
                        nc.vector.tensor_tensor(
                            oT[0:64, p, bass.ts(j, TT)], osb[0:64, :], rbs[:],
                            mybir.AluOpType.mult,
                        )
                    else:
                        onrm = stage.tile([64, TT], bf16, tag="onrm",
                                          name="onrm")
                        nc.vector.tensor_tensor(
                            onrm[:], osb[0:64, :], rbs[:], mybir.AluOpType.mult
                        )
                        nc.sync.dma_start(
                            oT[64:128, p, bass.ts(j, TT)], onrm[:]
                        )

            # scores PSUM: two parity tiles of 2 banks each.  Separate tiles
            # (not one [P,4,TT] tensor) so the tile-granular WAR dependency
            # lets sc(kc+2) overlap exp(kc+1): one tile would serialize every
            # score matmul behind the latest exp read, collapsing the
            # pipeline to 1-deep (measured 1.66us/kc vs ACT's 1.11us).
            big_scs = [ps_sc.tile([P, 2, TT], f32, tag=f"sc{par}",
                                  name=f"sc{par}") for par in range(2)]

            # ---- attention: ACT-paced kc pipeline with PE fillers ----
            pending_norm = None
            for j in range(NJ):
                for p in range(2):
                    o_ps = ps_acc.tile([P, 2, TT], f32, tag="acc", name="o_ps")

                    def sc_emit(kc, j=j, p=p):
                        sc = big_scs[kc % 2]
                        for i in range(2):
                            lo, hi = 64 * i, 64 * i + 64
                            nc.tensor.matmul(
                                sc[:, i, :],
                                kT[lo:hi, p, bass.ts(kc, P)],
                                qT[lo:hi, p, bass.ts(j, TT)],
                                start=True, stop=True,
                            )

                    def av_emit(kc, ex, p=p, o_ps=o_ps):
                        for i in range(2):
                            h = 2 * p + i
                            nc.tensor.matmul(
                                o_ps[0:VW, i, :],
                                vt[:, kc, bass.ds(VW * h, VW)],
                                ex[:, i, :],
                                start=(kc == 0), stop=(kc == NKT - 1),
                            )

                    sc_emit(0)
                    sc_emit(1)
                    if pending_norm is not None:
                        pending_norm()
                        pending_norm = None
                    prev = None
                    for kc in range(NKT):
                        ex = exps.tile([P, 2, TT], bf16, tag="exp", name="ex")
                        nc.scalar.activation(
                            ex[:], big_scs[kc % 2][:], Exp,
                            scale=0.125,
                        )
                        if prev is not None:
                            av_emit(kc - 1, prev)
                        if kc + 2 < NKT:
                            sc_emit(kc + 2)
                        drain_filler(NKT - kc)
                        prev = ex
                    av_emit(NKT - 1, prev)
                    osbs = norm_copies(o_ps)
                    pending_norm = (
                        lambda j=j, p=p, osbs=osbs: norm_arith(j, p, osbs)
                    )
                for oc in range(D // P):
                    fillers.append(outproj_group(j, oc))
            pending_norm()
            while fillers or active:
                drain_filler(NKT)

    nc.compile()
    return nc


def make_in_maps(x, Wq, bq, Wk, bk, Wv, Wo):
    import ml_dtypes

    bf = ml_dtypes.bfloat16
    xh = [np.ascontiguousarray(x[b].astype(bf)) for b in range(B)]

    in_maps = []
    for c in range(8):
        b, g = divmod(c, 4)
        cs = slice(COLS * g, COLS * (g + 1))
        in_maps.append({
            "xh": xh[b],
            "wq": np.ascontiguousarray(Wq[:, cs].astype(bf)),
            "wk": np.ascontiguousarray(Wk[:, cs].astype(bf)),
            "wv": np.ascontiguousarray(Wv[:, cs].astype(bf)),
            "wo": np.ascontiguousarray(Wo[cs, :].astype(bf)),
            "bq": np.ascontiguousarray(bq[cs]),
            "bk": np.ascontiguousarray(bk[cs]),
        })
    return in_maps


def kernel(x, Wq, bq, Wk, bk, Wv, bv, Wo, bo):
    from concourse import bass_utils

    x = np.asarray(x, dtype=np.float32)
    Wq = np.asarray(Wq, dtype=np.float32)
    Wk = np.asarray(Wk, dtype=np.float32)
    Wv = np.asarray(Wv, dtype=np.float32)
    Wo = np.asarray(Wo, dtype=np.float32)
    bq = np.asarray(bq, dtype=np.float32)
    bk = np.asarray(bk, dtype=np.float32)
    bv = np.asarray(bv, dtype=np.float32)
    bo = np.asarray(bo, dtype=np.float32)

    if "nc" not in _CACHE:
        _CACHE["nc"] = _build()
    nc = _CACHE["nc"]

    in_maps = make_in_maps(x, Wq, bq, Wk, bk, Wv, Wo)
    res = bass_utils.run_bass_kernel_spmd(nc, in_maps, core_ids=list(range(8)))

    out = np.zeros((B, S, D), dtype=np.float32)
    for c in range(8):
        out[c // 4] += res.results[c]["out_t"].T
    out += bo + bv @ Wo
    return out


# revision 13
# speedup vs baseline: 1.6254x; 1.0003x over previous
"""Multi-head attention (B=2, S=2048, D=1024, H=16, dk=64) on 8 Trainium2
NeuronCores via Bass/Tile.

Sharding: core c handles batch b = c//4 and head-group g = c%4 (4 heads,
256 qkv columns).  Each core computes its QKV projection slices, 4 heads of
attention, and a partial output projection against its 256-row slice of Wo.
The host sums the 4 partial outputs per batch (row-sharded Wo => partial
sums) and folds in the biases bo and bv@Wo (softmax rows sum to 1, so the
V-bias contributes exactly bv@Wo per token).

v3 design notes (vs v2's f32r):
- Everything bf16: fp32 feeds the PE at half the bf16 streaming rate
  (measured 1.2 rows/ns vs 2.4), so bf16 operands double matmul throughput.
  PSUM accumulation stays f32.  Measured numeric impact ~2.3e-3 rel_l2.
- Weights are cast to bf16 host-side -> straight DMA, no on-chip casts.
- x is DMA-transposed once (bf16, no hi/lo split).
- The exp on the ACT engine (~1.1us per [128,2,512] chunk) is the critical
  path of the attention phase.  Q-projections for blocks 1-3 and all output
  projections are emitted as "filler" PE work inside the attention kc loop,
  occupying PE stall slots under the ACT-bound pipeline instead of
  serializing before/after it.
- Normalization per head: PSUM->SBUF copy (bf16), PE ones-outer-product to
  broadcast the denominator row, DVE fast reciprocal, DVE multiply.  Head 0
  writes O^T directly; head 1 goes through one partition-shifting
  SBUF->SBUF DMA.
"""

import numpy as np

P = 128
B, S, D = 2, 2048, 1024
H, DK = 16, 64
COLS = 256          # qkv columns per core (4 heads)
KC = D // P         # 8 contraction chunks for the projections
TT = 512            # token block (matmul free dim)
NJ = S // TT        # 4 token blocks
NT = S // P         # 16 token tiles
NKT = S // P        # 16 key tiles
VW = 65             # per-head AV lhsT width: 64 v-dims + ones column

_CACHE = {}


def _build():
    import concourse.bass as bass
    import concourse.tile as tile
    from concourse import bacc, mybir

    f32 = mybir.dt.float32
    bf16 = mybir.dt.bfloat16
    Exp = mybir.ActivationFunctionType.Exp

    nc = bacc.Bacc(
        "TRN2", target_bir_lowering=False, debug=False,
        enable_asserts=False, num_devices=8,
    )
    xt_d = nc.dram_tensor("xt", [D, S], bf16, kind="ExternalInput").ap()
    wq_d = nc.dram_tensor("wq", [D, COLS], bf16, kind="ExternalInput").ap()
    wk_d = nc.dram_tensor("wk", [D, COLS], bf16, kind="ExternalInput").ap()
    wv_d = nc.dram_tensor("wv", [D, COLS], bf16, kind="ExternalInput").ap()
    wo_d = nc.dram_tensor("wo", [COLS, D], bf16, kind="ExternalInput").ap()
    bq_d = nc.dram_tensor("bq", [COLS], f32, kind="ExternalInput").ap()
    bk_d = nc.dram_tensor("bk", [COLS], f32, kind="ExternalInput").ap()
    out_d = nc.dram_tensor("out_t", [D, S], f32, kind="ExternalOutput").ap()

    with tile.TileContext(nc) as tc:
        with (
            tc.tile_pool(name="const", bufs=1) as const,
            tc.tile_pool(name="wpool", bufs=1) as wpool,
            tc.tile_pool(name="persist", bufs=1) as persist,
            tc.tile_pool(name="exps", bufs=3) as exps,
            tc.tile_pool(name="stage", bufs=4) as stage,
            tc.tile_pool(name="outst", bufs=4) as outst,
            tc.tile_pool(name="ps_sc", bufs=1, space="PSUM") as ps_sc,
            tc.tile_pool(name="ps_acc", bufs=1, space="PSUM") as ps_acc,
            tc.tile_pool(name="ps_u", bufs=2, space="PSUM") as ps_u,
        ):
            ones_b = const.tile([P, VW], bf16, tag="ones_b")
            nc.vector.memset(ones_b[:], 1.0)

            # ---- weights: already bf16 in DRAM (host-side cast); issued on
            # the scalar queue so they don't serialize behind the x DMAs on
            # the sync queue ----
            wk_r = wpool.tile([P, KC, COLS], bf16, tag="w_k")
            nc.scalar.dma_start(wk_r[:], wk_d.rearrange("(o p) f -> p o f", p=P))
            wv_r = wpool.tile([P, KC, COLS], bf16, tag="w_v")
            nc.scalar.dma_start(wv_r[:], wv_d.rearrange("(o p) f -> p o f", p=P))
            wq_r = wpool.tile([P, KC, COLS], bf16, tag="w_q")
            nc.scalar.dma_start(wq_r[:], wq_d.rearrange("(o p) f -> p o f", p=P))
            wo_r = wpool.tile([P, 2, D], bf16, tag="w_o")
            nc.scalar.dma_start(wo_r[:], wo_d.rearrange("(o p) f -> p o f", p=P))

            bq_sb = const.tile([P, 2], f32, tag="bq")
            nc.scalar.dma_start(bq_sb[:], bq_d.rearrange("(o p) -> p o", p=P))
            bk_sb = const.tile([P, 2], f32, tag="bk")
            nc.scalar.dma_start(bk_sb[:], bk_d.rearrange("(o p) -> p o", p=P))

            # persistent activations (all bf16)
            qT = persist.tile([P, 2, S], bf16, tag="qT")    # [qcol, tok]
            kT = persist.tile([P, 2, S], bf16, tag="kT")    # [kcol, tok]
            vt = persist.tile([P, NT, 4 * VW], bf16, tag="vt")  # [tok, h*(V|1)]
            oT = persist.tile([P, 2, S], bf16, tag="oT")    # [vdim, tok]
            xTs = [persist.tile([P, KC, TT], bf16, tag=f"xT{j}", name=f"xT{j}")
                   for j in range(NJ)]

            # ones column (index 64 of each head's VW slice)
            vt_heads = vt[:].rearrange("p t (h c) -> p t h c", c=VW)
            nc.vector.tensor_copy(
                vt_heads[:, :, :, 64],
                ones_b[:, :NT * 4].rearrange("p (t h) -> p t h", h=4),
            )

            xt_r = xt_d.rearrange("(o p) t -> p o t", p=P)
            for j in range(NJ):
                nc.sync.dma_start(xTs[j][:], xt_r[:, :, bass.ts(j, TT)])

            # ---- projection emitters ----
            def qk_proj_ct(j, wmat, bsb, dstT, ct, acc, kc0, kc1):
                for kc in range(kc0, kc1):
                    nc.tensor.matmul(
                        acc[:], wmat[:, kc, bass.ts(ct, P)], xTs[j][:, kc, :],
                        start=(kc == 0), stop=(kc == KC - 1),
                    )
                if kc1 == KC:
                    nc.vector.tensor_scalar_add(
                        dstT[:, ct, bass.ts(j, TT)], acc[:], bsb[:, ct : ct + 1]
                    )

            def v_proj(j, ts_):
                acc = ps_u.tile([P, COLS], f32, tag="u", name="v_acc")
                for kc in range(KC):
                    nc.tensor.matmul(
                        acc[:], xTs[j][:, kc, bass.ts(ts_, P)], wv_r[:, kc, :],
                        start=(kc == 0), stop=(kc == KC - 1),
                    )
                tt = 4 * j + ts_
                nc.vector.tensor_copy(
                    vt_heads[:, tt, :, 0:64],
                    acc[:].rearrange("p (h c) -> p h c", c=64),
                )

            # ---- phase A: all K and V projections + Q(0) up front ----
            for j in range(NJ):
                for ct in range(2):
                    acc = ps_u.tile([P, TT], f32, tag="u", name="k_acc")
                    qk_proj_ct(j, wk_r, bk_sb, kT, ct, acc, 0, KC)
                for ts_ in range(TT // P):
                    v_proj(j, ts_)
            for ct in range(2):
                acc = ps_u.tile([P, TT], f32, tag="u", name="q_acc")
                qk_proj_ct(0, wq_r, bq_sb, qT, ct, acc, 0, KC)

            # ---- filler queue: atomic groups of PE work (Q(1..3) column
            # tiles, out-projection units) drained one step per kc slot
            # inside the ACT-bound attention loop.  A multi-slot group is
            # only started when it fits in the current block's remaining
            # slots, so a ps_u accumulation never straddles the block
            # boundary where the norm's rbc tiles rotate through ps_u
            # (that interleaving could deadlock the in-order PE queue). ----
            fillers = []   # list of groups; group = list of step closures
            active = []    # remaining steps of the currently started group

            def qproj_group(j, ct):
                box = {}
                def step(kc0, box=box):
                    if kc0 == 0:
                        box["acc"] = ps_u.tile([P, TT], f32, tag="u",
                                               name="q_acc")
                    qk_proj_ct(j, wq_r, bq_sb, qT, ct, box["acc"],
                               kc0, kc0 + 2)
                return [lambda kc0=kc0: step(kc0) for kc0 in range(0, KC, 2)]

            def outproj_group(j, oc):
                def step():
                    acc = ps_u.tile([P, TT], f32, tag="u", name="wo_acc")
                    for vc in range(2):
                        nc.tensor.matmul(
                            acc[:], wo_r[:, vc, bass.ts(oc, P)],
                            oT[:, vc, bass.ts(j, TT)],
                            start=(vc == 0), stop=(vc == 1),
                        )
                    st = outst.tile([P, TT], f32, tag="outst", name="outst")
                    nc.vector.tensor_copy(st[:], acc[:])
                    nc.sync.dma_start(
                        out_d[bass.ts(oc, P), bass.ts(j, TT)], st[:]
                    )
                return [step]

            def drain_filler(slots_left):
                if not active:
                    for gi, grp in enumerate(fillers):
                        if len(grp) <= slots_left:
                            active.extend(fillers.pop(gi))
                            break
                    else:
                        return
                active.pop(0)()

            for j in range(1, NJ):
                for ct in range(2):
                    fillers.append(qproj_group(j, ct))

            # ---- normalization, split in two parts: the PSUM->SBUF copies
            # (the only o_ps reads) are emitted right after the last AV so
            # the next block's o_ps alloc records them; the arithmetic runs
            # after the next block's first scores so PE/ACT keep flowing ----
            def norm_copies(o_ps):
                osbs = []
                for i in range(2):
                    osb = stage.tile([VW, TT], bf16, tag="osb", name="osb")
                    nc.vector.tensor_copy(osb[:], o_ps[0:VW, i, :])
                    osbs.append(osb)
                return osbs

            def norm_arith(j, p, osbs):
                for i in range(2):
                    osb = osbs[i]
                    rbc = ps_u.tile([64, TT], f32, tag="u", name="rbc")
                    nc.tensor.matmul(
                        rbc[:], ones_b[64:65, 0:64], osb[64:65, :],
                        start=True, stop=True,
                    )
                    rbs = stage.tile([64, TT], f32, tag="rbs", name="rbs")
                    nc.vector.reciprocal_approx_fast(rbs[:], rbc[:])
                    if i == 0:
                        nc.vector.tensor_tensor(
                            oT[0:64, p, bass.ts(j, TT)], osb[0:64, :], rbs[:],
                            mybir.AluOpType.mult,
                        )
                    else:
                        onrm = stage.tile([64, TT], bf16, tag="onrm",
                                          name="onrm")
                        nc.vector.tensor_tensor(
                            onrm[:], osb[0:64, :], rbs[:], mybir.AluOpType.mult
                        )
                        nc.sync.dma_start(
                            oT[64:128, p, bass.ts(j, TT)], onrm[:]
                        )

            # scores PSUM: two parity tiles of 2 banks each.  Separate tiles
            # (not one [P,4,TT] tensor) so the tile-granular WAR dependency
            # lets sc(kc+2) overlap exp(kc+1): one tile would serialize every
            # score matmul behind the latest exp read, collapsing the
            # pipeline to 1-deep (measured 1.66us/kc vs ACT's 1.11us).
            big_scs = [ps_sc.tile([P, 2, TT], f32, tag=f"sc{par}",
                                  name=f"sc{par}") for par in range(2)]

            # ---- attention: ACT-paced kc pipeline with PE fillers.  The
            # next block's first two score pairs are emitted during the
            # current block's last two kc slots so the exp stream crosses
            # block boundaries without a bubble. ----
            blocks = [(j, p) for j in range(NJ) for p in range(2)]

            def sc_emit_b(t, kc):
                j, p = blocks[t]
                sc = big_scs[kc % 2]
                for i in range(2):
                    lo, hi = 64 * i, 64 * i + 64
                    nc.tensor.matmul(
                        sc[:, i, :],
                        kT[lo:hi, p, bass.ts(kc, P)],
                        qT[lo:hi, p, bass.ts(j, TT)],
                        start=True, stop=True,
                    )

            pending_norm = None
            for t, (j, p) in enumerate(blocks):
                o_ps = ps_acc.tile([P, 2, TT], f32, tag="acc", name="o_ps")

                def av_emit(kc, ex, p=p, o_ps=o_ps):
                    for i in range(2):
                        h = 2 * p + i
                        nc.tensor.matmul(
                            o_ps[0:VW, i, :],
                            vt[:, kc, bass.ds(VW * h, VW)],
                            ex[:, i, :],
                            start=(kc == 0), stop=(kc == NKT - 1),
                        )

                if t == 0:
                    sc_emit_b(0, 0)
                    sc_emit_b(0, 1)
                if pending_norm is not None:
                    pending_norm()
                    pending_norm = None
                prev = None
                for kc in range(NKT):
                    ex = exps.tile([P, 2, TT], bf16, tag="exp", name="ex")
                    nc.scalar.activation(
                        ex[:], big_scs[kc % 2][:], Exp, scale=0.125,
                    )
                    if prev is not None:
                        av_emit(kc - 1, prev)
                    if kc + 2 < NKT:
                        sc_emit_b(t, kc + 2)
                    elif t + 1 < len(blocks):
                        sc_emit_b(t + 1, kc - (NKT - 2))
                    drain_filler(NKT - kc)
                    prev = ex
                av_emit(NKT - 1, prev)
                osbs = norm_copies(o_ps)
                pending_norm = (
                    lambda j=j, p=p, osbs=osbs: norm_arith(j, p, osbs)
                )
                if p == 1:
                    for oc in range(D // P):
                        fillers.append(outproj_group(j, oc))
            pending_norm()
            while fillers or active:
                drain_filler(NKT)

    nc.compile()
    return nc


def make_in_maps(x, Wq, bq, Wk, bk, Wv, Wo):
    import ml_dtypes

    bf = ml_dtypes.bfloat16
    xt = [np.ascontiguousarray(x[b].T.astype(bf)) for b in range(B)]

    in_maps = []
    for c in range(8):
        b, g = divmod(c, 4)
        cs = slice(COLS * g, COLS * (g + 1))
        in_maps.append({
            "xt": xt[b],
            "wq": np.ascontiguousarray(Wq[:, cs].astype(bf)),
            "wk": np.ascontiguousarray(Wk[:, cs].astype(bf)),
            "wv": np.ascontiguousarray(Wv[:, cs].astype(bf)),
            "wo": np.ascontiguousarray(Wo[cs, :].astype(bf)),
            "bq": np.ascontiguousarray(bq[cs]),
            "bk": np.ascontiguousarray(bk[cs]),
        })
    return in_maps


def kernel(x, Wq, bq, Wk, bk, Wv, bv, Wo, bo):
    from concourse import bass_utils

    x = np.asarray(x, dtype=np.float32)
    Wq = np.asarray(Wq, dtype=np.float32)
    Wk = np.asarray(Wk, dtype=np.float32)
    Wv = np.asarray(Wv, dtype=np.float32)
    Wo = np.asarray(Wo, dtype=np.float32)
    bq = np.asarray(bq, dtype=np.float32)
    bk = np.asarray(bk, dtype=np.float32)
    bv = np.asarray(bv, dtype=np.float32)
    bo = np.asarray(bo, dtype=np.float32)

    if "nc" not in _CACHE:
        _CACHE["nc"] = _build()
    nc = _CACHE["nc"]

    in_maps = make_in_maps(x, Wq, bq, Wk, bk, Wv, Wo)
    res = bass_utils.run_bass_kernel_spmd(nc, in_maps, core_ids=list(range(8)))

    out = np.zeros((B, S, D), dtype=np.float32)
    for c in range(8):
        out[c // 4] += res.results[c]["out_t"].T
    out += bo + bv @ Wo
    return out


# revision 24
# speedup vs baseline: 1.6553x; 1.0183x over previous
"""Multi-head attention (B=2, S=2048, D=1024, H=16, dk=64) on 8 Trainium2
NeuronCores via Bass/Tile.

Sharding: core c handles batch b = c//4 and head-group g = c%4 (4 heads,
256 qkv columns).  Each core computes its QKV projection slices, 4 heads of
attention, and a partial output projection against its 256-row slice of Wo.
The host sums the 4 partial outputs per batch (row-sharded Wo => partial
sums) and folds in the biases bo and bv@Wo (softmax rows sum to 1, so the
V-bias contributes exactly bv@Wo per token).

v3 design notes (vs v2's f32r):
- Everything bf16: fp32 feeds the PE at half the bf16 streaming rate
  (measured 1.2 rows/ns vs 2.4), so bf16 operands double matmul throughput.
  PSUM accumulation stays f32.  Measured numeric impact ~2.3e-3 rel_l2.
- Weights are cast to bf16 host-side -> straight DMA, no on-chip casts.
- x is DMA-transposed once (bf16, no hi/lo split).
- The exp on the ACT engine (~1.1us per [128,2,512] chunk) is the critical
  path of the attention phase.  Q-projections for blocks 1-3 and all output
  projections are emitted as "filler" PE work inside the attention kc loop,
  occupying PE stall slots under the ACT-bound pipeline instead of
  serializing before/after it.
- Normalization per head: PSUM->SBUF copy (bf16), PE ones-outer-product to
  broadcast the denominator row, DVE fast reciprocal, DVE multiply.  Head 0
  writes O^T directly; head 1 goes through one partition-shifting
  SBUF->SBUF DMA.
"""

import numpy as np

P = 128
B, S, D = 2, 2048, 1024
H, DK = 16, 64
COLS = 256          # qkv columns per core (4 heads)
KC = D // P         # 8 contraction chunks for the projections
TT = 512            # token block (matmul free dim)
NJ = S // TT        # 4 token blocks
NT = S // P         # 16 token tiles
NKT = S // P        # 16 key tiles
VW = 65             # per-head AV lhsT width: 64 v-dims + ones column

_CACHE = {}


def _build():
    import concourse.bass as bass
    import concourse.tile as tile
    from concourse import bacc, mybir

    f32 = mybir.dt.float32
    bf16 = mybir.dt.bfloat16
    Exp = mybir.ActivationFunctionType.Exp

    nc = bacc.Bacc(
        "TRN2", target_bir_lowering=False, debug=False,
        enable_asserts=False, num_devices=8,
    )
    xt_d = nc.dram_tensor("xt", [D, S], bf16, kind="ExternalInput").ap()
    wq_d = nc.dram_tensor("wq", [D, COLS], bf16, kind="ExternalInput").ap()
    wk_d = nc.dram_tensor("wk", [D, COLS], bf16, kind="ExternalInput").ap()
    wv_d = nc.dram_tensor("wv", [D, COLS], bf16, kind="ExternalInput").ap()
    wo_d = nc.dram_tensor("wo", [COLS, D], bf16, kind="ExternalInput").ap()
    bq_d = nc.dram_tensor("bq", [P, 2], f32, kind="ExternalInput").ap()
    bk_d = nc.dram_tensor("bk", [P, 2], f32, kind="ExternalInput").ap()
    out_d = nc.dram_tensor("out_t", [D, S], f32, kind="ExternalOutput").ap()

    with tile.TileContext(nc) as tc:
        with (
            tc.tile_pool(name="const", bufs=1) as const,
            tc.tile_pool(name="wpool", bufs=1) as wpool,
            tc.tile_pool(name="persist", bufs=1) as persist,
            tc.tile_pool(name="exps", bufs=3) as exps,
            tc.tile_pool(name="stage", bufs=4) as stage,
            tc.tile_pool(name="outst", bufs=4) as outst,
            tc.tile_pool(name="ps_sc", bufs=1, space="PSUM") as ps_sc,
            tc.tile_pool(name="ps_acc", bufs=1, space="PSUM") as ps_acc,
            tc.tile_pool(name="ps_u", bufs=2, space="PSUM") as ps_u,
        ):
            ones_b = const.tile([P, VW], bf16, tag="ones_b")
            nc.vector.memset(ones_b[:], 1.0)

            # ---- weights: already bf16 in DRAM (host-side cast); issued on
            # the scalar queue so they don't serialize behind the x DMAs on
            # the sync queue.  Biases come pre-shaped [128, 2] from the host
            # (a "(o p) -> p o" DRAM gather is 256 4-byte descriptors that
            # crawl through the DMA fabric) and load first — the first
            # K bias-add otherwise stalls the whole projection chain. ----
            bq_sb = const.tile([P, 2], f32, tag="bq")
            nc.scalar.dma_start(bq_sb[:], bq_d)
            bk_sb = const.tile([P, 2], f32, tag="bk")
            nc.scalar.dma_start(bk_sb[:], bk_d)
            wk_r = wpool.tile([P, KC, COLS], bf16, tag="w_k")
            nc.scalar.dma_start(wk_r[:], wk_d.rearrange("(o p) f -> p o f", p=P))
            wv_r = wpool.tile([P, KC, COLS], bf16, tag="w_v")
            nc.scalar.dma_start(wv_r[:], wv_d.rearrange("(o p) f -> p o f", p=P))
            wq_r = wpool.tile([P, KC, COLS], bf16, tag="w_q")
            nc.scalar.dma_start(wq_r[:], wq_d.rearrange("(o p) f -> p o f", p=P))
            wo_r = wpool.tile([P, 2, D], bf16, tag="w_o")
            nc.scalar.dma_start(wo_r[:], wo_d.rearrange("(o p) f -> p o f", p=P))

            # persistent activations (all bf16)
            qT = persist.tile([P, 2, S], bf16, tag="qT")    # [qcol, tok]
            kT = persist.tile([P, 2, S], bf16, tag="kT")    # [kcol, tok]
            vt = persist.tile([P, NT, 4 * VW], bf16, tag="vt")  # [tok, h*(V|1)]
            oT = persist.tile([P, 2, S], bf16, tag="oT")    # [vdim, tok]
            xTs = [persist.tile([P, KC, TT], bf16, tag=f"xT{j}", name=f"xT{j}")
                   for j in range(NJ)]

            # ones column (index 64 of each head's VW slice)
            vt_heads = vt[:].rearrange("p t (h c) -> p t h c", c=VW)
            nc.vector.tensor_copy(
                vt_heads[:, :, :, 64],
                ones_b[:, :NT * 4].rearrange("p (t h) -> p t h", h=4),
            )

            xt_r = xt_d.rearrange("(o p) t -> p o t", p=P)
            for j in range(NJ):
                nc.sync.dma_start(xTs[j][:], xt_r[:, :, bass.ts(j, TT)])

            # ---- projection emitters ----
            def qk_proj_ct(j, wmat, bsb, dstT, ct, acc, kc0, kc1):
                for kc in range(kc0, kc1):
                    nc.tensor.matmul(
                        acc[:], wmat[:, kc, bass.ts(ct, P)], xTs[j][:, kc, :],
                        start=(kc == 0), stop=(kc == KC - 1),
                    )
                if kc1 == KC:
                    nc.vector.tensor_scalar_add(
                        dstT[:, ct, bass.ts(j, TT)], acc[:], bsb[:, ct : ct + 1]
                    )

            def v_proj(j, ts_):
                acc = ps_u.tile([P, COLS], f32, tag="u", name="v_acc")
                for kc in range(KC):
                    nc.tensor.matmul(
                        acc[:], xTs[j][:, kc, bass.ts(ts_, P)], wv_r[:, kc, :],
                        start=(kc == 0), stop=(kc == KC - 1),
                    )
                tt = 4 * j + ts_
                nc.vector.tensor_copy(
                    vt_heads[:, tt, :, 0:64],
                    acc[:].rearrange("p (h c) -> p h c", c=64),
                )

            # ---- phase A (lead-in): only what block (0,0) needs up front:
            # K(0) ct0, V(0), Q(0) ct0.  Everything else becomes PE filler
            # work inside the ACT-paced attention loop — PE is the global
            # bottleneck, so projection work must hide under the exp
            # stream instead of serializing before it. ----
            for ct in (0,):
                acc = ps_u.tile([P, TT], f32, tag="u", name="k_acc")
                qk_proj_ct(0, wk_r, bk_sb, kT, ct, acc, 0, KC)
            for ts_ in range(TT // P):
                v_proj(0, ts_)
            acc = ps_u.tile([P, TT], f32, tag="u", name="q_acc")
            qk_proj_ct(0, wq_r, bq_sb, qT, 0, acc, 0, KC)

            # ---- filler queue: atomic groups of PE work (projection column
            # tiles, V units, out-projection units) drained a few steps per
            # kc slot inside the attention loop.  A multi-slot group is only
            # started when it fits in the current block's remaining slots,
            # so a ps_u accumulation never straddles the block boundary
            # where the norm's rbc tiles rotate through ps_u (that
            # interleaving could deadlock the in-order PE queue). ----
            fillers = []      # list of (key, [step closures])
            active = []       # remaining steps of the started group
            active_key = [None]
            done_keys = set()
            # produced in the lead-in:
            done_keys.update([("k", 0, 0), ("q", 0, 0)])
            done_keys.update([("v", 0, ts_) for ts_ in range(4)])

            def qkproj_group(j, ct, wmat, bsb, dstT, nm):
                box = {}
                def step(kc0, box=box):
                    if kc0 == 0:
                        box["acc"] = ps_u.tile([P, TT], f32, tag="u", name=nm)
                    qk_proj_ct(j, wmat, bsb, dstT, ct, box["acc"],
                               kc0, kc0 + 2)
                return [lambda kc0=kc0: step(kc0) for kc0 in range(0, KC, 2)]

            def vproj_group(j, ts_):
                box = {}
                def step(kc0, box=box):
                    if kc0 == 0:
                        box["acc"] = ps_u.tile([P, COLS], f32, tag="u",
                                               name="v_acc")
                    acc = box["acc"]
                    for kc in range(kc0, kc0 + 4):
                        nc.tensor.matmul(
                            acc[:], xTs[j][:, kc, bass.ts(ts_, P)],
                            wv_r[:, kc, :],
                            start=(kc == 0), stop=(kc == KC - 1),
                        )
                    if kc0 == 4:
                        tt = 4 * j + ts_
                        nc.vector.tensor_copy(
                            vt_heads[:, tt, :, 0:64],
                            acc[:].rearrange("p (h c) -> p h c", c=64),
                        )
                return [lambda kc0=kc0: step(kc0) for kc0 in (0, 4)]

            def outproj_group(j, oc):
                def step():
                    acc = ps_u.tile([P, TT], f32, tag="u", name="wo_acc")
                    for vc in range(2):
                        nc.tensor.matmul(
                            acc[:], wo_r[:, vc, bass.ts(oc, P)],
                            oT[:, vc, bass.ts(j, TT)],
                            start=(vc == 0), stop=(vc == 1),
                        )
                    st = outst.tile([P, TT], f32, tag="outst", name="outst")
                    nc.vector.tensor_copy(st[:], acc[:])
                    nc.sync.dma_start(
                        out_d[bass.ts(oc, P), bass.ts(j, TT)], st[:]
                    )
                return [step]

            def _finish_active():
                while active:
                    active.pop(0)()
                if active_key[0] is not None:
                    done_keys.add(active_key[0])
                    active_key[0] = None

            def drain_filler(slots_left, n=1):
                for _ in range(n):
                    if not active:
                        if active_key[0] is not None:
                            done_keys.add(active_key[0])
                            active_key[0] = None
                        for gi, (key, grp) in enumerate(fillers):
                            if len(grp) <= slots_left:
                                key, grp = fillers.pop(gi)
                                active.extend(grp)
                                active_key[0] = key
                                break
                        else:
                            return
                    active.pop(0)()
                if not active and active_key[0] is not None:
                    done_keys.add(active_key[0])
                    active_key[0] = None

            def ensure(key):
                # force-emit producer groups (in queue order) until `key`
                # has been fully emitted.  Called before the consumer is
                # emitted so the dependency is recorded.
                if key in done_keys:
                    return
                if active_key[0] == key:
                    _finish_active()
                    return
                while key not in done_keys:
                    _finish_active()
                    if not fillers:
                        raise RuntimeError(f"missing producer {key}")
                    k, grp = fillers.pop(0)
                    active.extend(grp)
                    active_key[0] = k
                _finish_active()

            # production order: per j, the K/Q ct0 and V needed by the p=0
            # blocks; then all ct1 work needed by the p=1 blocks.
            for j in range(1, NJ):
                fillers.append((("k", j, 0),
                                qkproj_group(j, 0, wk_r, bk_sb, kT, "k_acc")))
                fillers.append((("q", j, 0),
                                qkproj_group(j, 0, wq_r, bq_sb, qT, "q_acc")))
                for ts_ in range(TT // P):
                    fillers.append((("v", j, ts_), vproj_group(j, ts_)))
            for j in range(NJ):
                fillers.append((("k", j, 1),
                                qkproj_group(j, 1, wk_r, bk_sb, kT, "k_acc")))
            for j in range(NJ):
                fillers.append((("q", j, 1),
                                qkproj_group(j, 1, wq_r, bq_sb, qT, "q_acc")))

            # ---- normalization, split in two parts: the PSUM->SBUF copies
            # (the only o_ps reads) are emitted right after the last AV so
            # the next block's o_ps alloc records them; the arithmetic runs
            # after the next block's first scores so PE/ACT keep flowing ----
            def norm_copies(o_ps):
                osbs = []
                for i in range(2):
                    osb = stage.tile([VW, TT], bf16, tag="osb", name="osb")
                    nc.vector.tensor_copy(osb[:], o_ps[0:VW, i, :])
                    osbs.append(osb)
                return osbs

            def norm_arith(j, p, osbs):
                for i in range(2):
                    osb = osbs[i]
                    rbc = ps_u.tile([64, TT], f32, tag="u", name="rbc")
                    nc.tensor.matmul(
                        rbc[:], ones_b[64:65, 0:64], osb[64:65, :],
                        start=True, stop=True,
                    )
                    rbs = stage.tile([64, TT], f32, tag="rbs", name="rbs")
                    nc.vector.reciprocal_approx_fast(rbs[:], rbc[:])
                    if i == 0:
                        nc.vector.tensor_tensor(
                            oT[0:64, p, bass.ts(j, TT)], osb[0:64, :], rbs[:],
                            mybir.AluOpType.mult,
                        )
                    else:
                        onrm = stage.tile([64, TT], bf16, tag="onrm",
                                          name="onrm")
                        nc.vector.tensor_tensor(
                            onrm[:], osb[0:64, :], rbs[:], mybir.AluOpType.mult
                        )
                        nc.sync.dma_start(
                            oT[64:128, p, bass.ts(j, TT)], onrm[:]
                        )

            # scores PSUM: two parity tiles of 2 banks each.  Separate tiles
            # (not one [P,4,TT] tensor) so the tile-granular WAR dependency
            # lets sc(kc+2) overlap exp(kc+1): one tile would serialize every
            # score matmul behind the latest exp read, collapsing the
            # pipeline to 1-deep (measured 1.66us/kc vs ACT's 1.11us).
            big_scs = [ps_sc.tile([P, 2, TT], f32, tag=f"sc{par}",
                                  name=f"sc{par}") for par in range(2)]

            # ---- attention: ACT-paced kc pipeline with PE fillers.  The
            # next block's first two score pairs are emitted during the
            # current block's last two kc slots so the exp stream crosses
            # block boundaries without a bubble. ----
            blocks = [(j, p) for p in range(2) for j in range(NJ)]

            def sc_emit_b(t, kc):
                j, p = blocks[t]
                ensure(("k", kc // 4, p))
                ensure(("q", j, p))
                sc = big_scs[kc % 2]
                for i in range(2):
                    lo, hi = 64 * i, 64 * i + 64
                    nc.tensor.matmul(
                        sc[:, i, :],
                        kT[lo:hi, p, bass.ts(kc, P)],
                        qT[lo:hi, p, bass.ts(j, TT)],
                        start=True, stop=True,
                    )

            pending_norm = None
            for t, (j, p) in enumerate(blocks):
                o_ps = ps_acc.tile([P, 2, TT], f32, tag="acc", name="o_ps")

                def av_emit(kc, ex, p=p, o_ps=o_ps):
                    ensure(("v", kc // 4, kc % 4))
                    for i in range(2):
                        h = 2 * p + i
                        nc.tensor.matmul(
                            o_ps[0:VW, i, :],
                            vt[:, kc, bass.ds(VW * h, VW)],
                            ex[:, i, :],
                            start=(kc == 0), stop=(kc == NKT - 1),
                        )

                if t == 0:
                    sc_emit_b(0, 0)
                    sc_emit_b(0, 1)
                if pending_norm is not None:
                    pending_norm()
                    pending_norm = None
                ndrain = 3 if t == 0 else (2 if t < NJ else 1)
                prev = None
                for kc in range(NKT):
                    ex = exps.tile([P, 2, TT], bf16, tag="exp", name="ex")
                    nc.scalar.activation(
                        ex[:], big_scs[kc % 2][:], Exp, scale=0.125,
                    )
                    drain_filler(NKT - kc, ndrain)
                    if prev is not None:
                        av_emit(kc - 1, prev)
                    if kc + 2 < NKT:
                        sc_emit_b(t, kc + 2)
                    elif t + 1 < len(blocks):
                        sc_emit_b(t + 1, kc - (NKT - 2))
                    prev = ex
                av_emit(NKT - 1, prev)
                osbs = norm_copies(o_ps)
                pending_norm = (
                    lambda j=j, p=p, osbs=osbs: norm_arith(j, p, osbs)
                )
                if p == 1:
                    for oc in range(D // P):
                        fillers.append((("o", j, oc), outproj_group(j, oc)))
            pending_norm()
            while fillers or active:
                drain_filler(NKT)

    nc.compile()
    return nc


def make_in_maps(x, Wq, bq, Wk, bk, Wv, Wo):
    import ml_dtypes

    bf = ml_dtypes.bfloat16
    xt = [np.ascontiguousarray(x[b].T.astype(bf)) for b in range(B)]

    in_maps = []
    for c in range(8):
        b, g = divmod(c, 4)
        cs = slice(COLS * g, COLS * (g + 1))
        in_maps.append({
            "xt": xt[b],
            "wq": np.ascontiguousarray(Wq[:, cs].astype(bf)),
            "wk": np.ascontiguousarray(Wk[:, cs].astype(bf)),
            "wv": np.ascontiguousarray(Wv[:, cs].astype(bf)),
            "wo": np.ascontiguousarray(Wo[cs, :].astype(bf)),
            "bq": np.ascontiguousarray(bq[cs].reshape(2, P).T),
            "bk": np.ascontiguousarray(bk[cs].reshape(2, P).T),
        })
    return in_maps


def kernel(x, Wq, bq, Wk, bk, Wv, bv, Wo, bo):
    from concourse import bass_utils

    x = np.asarray(x, dtype=np.float32)
    Wq = np.asarray(Wq, dtype=np.float32)
    Wk = np.asarray(Wk, dtype=np.float32)
    Wv = np.asarray(Wv, dtype=np.float32)
    Wo = np.asarray(Wo, dtype=np.float32)
    bq = np.asarray(bq, dtype=np.float32)
    bk = np.asarray(bk, dtype=np.float32)
    bv = np.asarray(bv, dtype=np.float32)
    bo = np.asarray(bo, dtype=np.float32)

    if "nc" not in _CACHE:
        _CACHE["nc"] = _build()
    nc = _CACHE["nc"]

    in_maps = make_in_maps(x, Wq, bq, Wk, bk, Wv, Wo)
    res = bass_utils.run_bass_kernel_spmd(nc, in_maps, core_ids=list(range(8)))

    out = np.zeros((B, S, D), dtype=np.float32)
    for c in range(8):
        out[c // 4] += res.results[c]["out_t"].T
    out += bo + bv @ Wo
    return out


# revision 29
# speedup vs baseline: 1.7306x; 1.0455x over previous
"""Multi-head attention (B=2, S=2048, D=1024, H=16, dk=64) on 8 Trainium2
NeuronCores via Bass/Tile.

Sharding: core c handles batch b = c//4 and head-group g = c%4 (4 heads,
256 qkv columns).  Each core computes its QKV projection slices, 4 heads of
attention, and a partial output projection against its 256-row slice of Wo.
The host sums the 4 partial outputs per batch (row-sharded Wo => partial
sums) and folds in the biases bo and bv@Wo (softmax rows sum to 1, so the
V-bias contributes exactly bv@Wo per token).

v3 design notes (vs v2's f32r):
- Everything bf16: fp32 feeds the PE at half the bf16 streaming rate
  (measured 1.2 rows/ns vs 2.4), so bf16 operands double matmul throughput.
  PSUM accumulation stays f32.  Measured numeric impact ~2.3e-3 rel_l2.
- Weights are cast to bf16 host-side -> straight DMA, no on-chip casts.
- x is DMA-transposed once (bf16, no hi/lo split).
- The exp on the ACT engine (~1.1us per [128,2,512] chunk) is the critical
  path of the attention phase.  Q-projections for blocks 1-3 and all output
  projections are emitted as "filler" PE work inside the attention kc loop,
  occupying PE stall slots under the ACT-bound pipeline instead of
  serializing before/after it.
- Normalization per head: PSUM->SBUF copy (bf16), PE ones-outer-product to
  broadcast the denominator row, DVE fast reciprocal, DVE multiply.  Head 0
  writes O^T directly; head 1 goes through one partition-shifting
  SBUF->SBUF DMA.
"""

import numpy as np

P = 128
B, S, D = 2, 2048, 1024
H, DK = 16, 64
COLS = 256          # qkv columns per core (4 heads)
KC = D // P         # 8 contraction chunks for the projections
TT = 512            # token block (matmul free dim)
NJ = S // TT        # 4 token blocks
NT = S // P         # 16 token tiles
NKT = S // P        # 16 key tiles
VW = 65             # per-head AV lhsT width: 64 v-dims + ones column

_CACHE = {}


def _build():
    import concourse.bass as bass
    import concourse.tile as tile
    from concourse import bacc, mybir

    f32 = mybir.dt.float32
    bf16 = mybir.dt.bfloat16
    Exp = mybir.ActivationFunctionType.Exp

    nc = bacc.Bacc(
        "TRN2", target_bir_lowering=False, debug=False,
        enable_asserts=False, num_devices=8,
    )
    xt_d = nc.dram_tensor("xt", [D, S], bf16, kind="ExternalInput").ap()
    wq_d = nc.dram_tensor("wq", [D, COLS], bf16, kind="ExternalInput").ap()
    wk_d = nc.dram_tensor("wk", [D, COLS], bf16, kind="ExternalInput").ap()
    wv_d = nc.dram_tensor("wv", [D, COLS], bf16, kind="ExternalInput").ap()
    wo_d = nc.dram_tensor("wo", [COLS, D], bf16, kind="ExternalInput").ap()
    bq_d = nc.dram_tensor("bq", [P, 2], f32, kind="ExternalInput").ap()
    bk_d = nc.dram_tensor("bk", [P, 2], f32, kind="ExternalInput").ap()
    out_d = nc.dram_tensor("out_t", [D, S], bf16, kind="ExternalOutput").ap()

    with tile.TileContext(nc) as tc:
        with (
            tc.tile_pool(name="const", bufs=1) as const,
            tc.tile_pool(name="wpool", bufs=1) as wpool,
            tc.tile_pool(name="persist", bufs=1) as persist,
            tc.tile_pool(name="exps", bufs=3) as exps,
            tc.tile_pool(name="stage", bufs=4) as stage,
            tc.tile_pool(name="outst", bufs=4) as outst,
            tc.tile_pool(name="ps_sc", bufs=1, space="PSUM") as ps_sc,
            tc.tile_pool(name="ps_acc", bufs=1, space="PSUM") as ps_acc,
            tc.tile_pool(name="ps_u", bufs=2, space="PSUM") as ps_u,
        ):
            ones_b = const.tile([P, VW], bf16, tag="ones_b")
            nc.vector.memset(ones_b[:], 1.0)

            # ---- weights: already bf16 in DRAM (host-side cast); issued on
            # the scalar queue so they don't serialize behind the x DMAs on
            # the sync queue.  Biases come pre-shaped [128, 2] from the host
            # (a "(o p) -> p o" DRAM gather is 256 4-byte descriptors that
            # crawl through the DMA fabric) and load first — the first
            # K bias-add otherwise stalls the whole projection chain. ----
            wk_r = wpool.tile([P, KC, COLS], bf16, tag="w_k")
            nc.scalar.dma_start(wk_r[:], wk_d.rearrange("(o p) f -> p o f", p=P))
            bq_sb = const.tile([P, 2], f32, tag="bq")
            nc.scalar.dma_start(bq_sb[:], bq_d)
            bk_sb = const.tile([P, 2], f32, tag="bk")
            nc.scalar.dma_start(bk_sb[:], bk_d)
            wv_r = wpool.tile([P, KC, COLS], bf16, tag="w_v")
            nc.scalar.dma_start(wv_r[:], wv_d.rearrange("(o p) f -> p o f", p=P))
            wq_r = wpool.tile([P, KC, COLS], bf16, tag="w_q")
            nc.scalar.dma_start(wq_r[:], wq_d.rearrange("(o p) f -> p o f", p=P))
            wo_r = wpool.tile([P, 2, D], bf16, tag="w_o")
            nc.scalar.dma_start(wo_r[:], wo_d.rearrange("(o p) f -> p o f", p=P))

            # persistent activations (all bf16)
            qT = persist.tile([P, 2, S], bf16, tag="qT")    # [qcol, tok]
            kT = persist.tile([P, 2, S], bf16, tag="kT")    # [kcol, tok]
            vt = persist.tile([P, NT, 4 * VW], bf16, tag="vt")  # [tok, h*(V|1)]
            oT = persist.tile([P, 2, S], bf16, tag="oT")    # [vdim, tok]
            xTs = [persist.tile([P, KC, TT], bf16, tag=f"xT{j}", name=f"xT{j}")
                   for j in range(NJ)]

            # ones column (index 64 of each head's VW slice)
            vt_heads = vt[:].rearrange("p t (h c) -> p t h c", c=VW)
            nc.vector.tensor_copy(
                vt_heads[:, :, :, 64],
                ones_b[:, :NT * 4].rearrange("p (t h) -> p t h", h=4),
            )

            xt_r = xt_d.rearrange("(o p) t -> p o t", p=P)
            for j in range(NJ):
                nc.sync.dma_start(xTs[j][:], xt_r[:, :, bass.ts(j, TT)])

            # ---- projection emitters ----
            def qk_proj_ct(j, wmat, bsb, dstT, ct, acc, kc0, kc1):
                for kc in range(kc0, kc1):
                    nc.tensor.matmul(
                        acc[:], wmat[:, kc, bass.ts(ct, P)], xTs[j][:, kc, :],
                        start=(kc == 0), stop=(kc == KC - 1),
                    )
                if kc1 == KC:
                    nc.vector.tensor_scalar_add(
                        dstT[:, ct, bass.ts(j, TT)], acc[:], bsb[:, ct : ct + 1]
                    )

            def v_proj(j, ts_):
                acc = ps_u.tile([P, COLS], f32, tag="u", name="v_acc")
                for kc in range(KC):
                    nc.tensor.matmul(
                        acc[:], xTs[j][:, kc, bass.ts(ts_, P)], wv_r[:, kc, :],
                        start=(kc == 0), stop=(kc == KC - 1),
                    )
                tt = 4 * j + ts_
                nc.vector.tensor_copy(
                    vt_heads[:, tt, :, 0:64],
                    acc[:].rearrange("p (h c) -> p h c", c=64),
                )

            # ---- phase A (lead-in): only what block (0,0) needs up front:
            # K(0) ct0, V(0), Q(0) ct0.  Everything else becomes PE filler
            # work inside the ACT-paced attention loop — PE is the global
            # bottleneck, so projection work must hide under the exp
            # stream instead of serializing before it. ----
            for ct in (0,):
                acc = ps_u.tile([P, TT], f32, tag="u", name="k_acc")
                qk_proj_ct(0, wk_r, bk_sb, kT, ct, acc, 0, KC)
            for ts_ in range(TT // P):
                v_proj(0, ts_)
            acc = ps_u.tile([P, TT], f32, tag="u", name="q_acc")
            qk_proj_ct(0, wq_r, bq_sb, qT, 0, acc, 0, KC)

            # ---- filler queue: atomic groups of PE work (projection column
            # tiles, V units, out-projection units) drained a few steps per
            # kc slot inside the attention loop.  A multi-slot group is only
            # started when it fits in the current block's remaining slots,
            # so a ps_u accumulation never straddles the block boundary
            # where the norm's rbc tiles rotate through ps_u (that
            # interleaving could deadlock the in-order PE queue). ----
            fillers = []      # list of (key, [step closures])
            active = []       # remaining steps of the started group
            active_key = [None]
            done_keys = set()
            # produced in the lead-in:
            done_keys.update([("k", 0, 0), ("q", 0, 0)])
            done_keys.update([("v", 0, ts_) for ts_ in range(4)])

            def qkproj_group(j, ct, wmat, bsb, dstT, nm):
                box = {}
                def step(kc0, box=box):
                    if kc0 == 0:
                        box["acc"] = ps_u.tile([P, TT], f32, tag="u", name=nm)
                    qk_proj_ct(j, wmat, bsb, dstT, ct, box["acc"],
                               kc0, kc0 + 2)
                return [lambda kc0=kc0: step(kc0) for kc0 in range(0, KC, 2)]

            def vproj_group(j, ts_):
                box = {}
                def step(kc0, box=box):
                    if kc0 == 0:
                        box["acc"] = ps_u.tile([P, COLS], f32, tag="u",
                                               name="v_acc")
                    acc = box["acc"]
                    for kc in range(kc0, kc0 + 4):
                        nc.tensor.matmul(
                            acc[:], xTs[j][:, kc, bass.ts(ts_, P)],
                            wv_r[:, kc, :],
                            start=(kc == 0), stop=(kc == KC - 1),
                        )
                    if kc0 == 4:
                        tt = 4 * j + ts_
                        nc.vector.tensor_copy(
                            vt_heads[:, tt, :, 0:64],
                            acc[:].rearrange("p (h c) -> p h c", c=64),
                        )
                return [lambda kc0=kc0: step(kc0) for kc0 in (0, 4)]

            def outproj_group(j, oc):
                def step():
                    acc = ps_u.tile([P, TT], f32, tag="u", name="wo_acc")
                    for vc in range(2):
                        nc.tensor.matmul(
                            acc[:], wo_r[:, vc, bass.ts(oc, P)],
                            oT[:, vc, bass.ts(j, TT)],
                            start=(vc == 0), stop=(vc == 1),
                        )
                    st = outst.tile([P, TT], bf16, tag="outst", name="outst")
                    if j == NJ - 1:
                        # the last j's units drain after the final exp: use
                        # the then-idle ACT engine so the tail's PSUM->SBUF
                        # copies don't serialize on DVE behind the norm
                        nc.scalar.copy(st[:], acc[:])
                    else:
                        nc.vector.tensor_copy(st[:], acc[:])
                    nc.sync.dma_start(
                        out_d[bass.ts(oc, P), bass.ts(j, TT)], st[:]
                    )
                return [step]

            def _finish_active():
                while active:
                    active.pop(0)()
                if active_key[0] is not None:
                    done_keys.add(active_key[0])
                    active_key[0] = None

            def drain_filler(slots_left, n=1):
                for _ in range(n):
                    if not active:
                        if active_key[0] is not None:
                            done_keys.add(active_key[0])
                            active_key[0] = None
                        for gi, (key, grp) in enumerate(fillers):
                            if len(grp) <= slots_left:
                                key, grp = fillers.pop(gi)
                                active.extend(grp)
                                active_key[0] = key
                                break
                        else:
                            return
                    active.pop(0)()
                if not active and active_key[0] is not None:
                    done_keys.add(active_key[0])
                    active_key[0] = None

            def ensure(key):
                # force-emit producer groups (in queue order) until `key`
                # has been fully emitted.  Called before the consumer is
                # emitted so the dependency is recorded.
                if key in done_keys:
                    return
                if active_key[0] == key:
                    _finish_active()
                    return
                while key not in done_keys:
                    _finish_active()
                    if not fillers:
                        raise RuntimeError(f"missing producer {key}")
                    k, grp = fillers.pop(0)
                    active.extend(grp)
                    active_key[0] = k
                _finish_active()

            # production order: per j, the K/Q ct0 and V needed by the p=0
            # blocks; then all ct1 work needed by the p=1 blocks.
            for j in range(1, NJ):
                fillers.append((("k", j, 0),
                                qkproj_group(j, 0, wk_r, bk_sb, kT, "k_acc")))
                fillers.append((("q", j, 0),
                                qkproj_group(j, 0, wq_r, bq_sb, qT, "q_acc")))
                for ts_ in range(TT // P):
                    fillers.append((("v", j, ts_), vproj_group(j, ts_)))
            for j in range(NJ):
                fillers.append((("k", j, 1),
                                qkproj_group(j, 1, wk_r, bk_sb, kT, "k_acc")))
            for j in range(NJ):
                fillers.append((("q", j, 1),
                                qkproj_group(j, 1, wq_r, bq_sb, qT, "q_acc")))

            # ---- normalization, split in two parts: the PSUM->SBUF copies
            # (the only o_ps reads) are emitted right after the last AV so
            # the next block's o_ps alloc records them; the arithmetic runs
            # after the next block's first scores so PE/ACT keep flowing ----
            def norm_copies(o_ps):
                osbs = []
                for i in range(2):
                    osb = stage.tile([VW, TT], bf16, tag="osb", name="osb")
                    nc.vector.tensor_copy(osb[:], o_ps[0:VW, i, :])
                    osbs.append(osb)
                return osbs

            def norm_arith(j, p, osbs):
                for i in range(2):
                    osb = osbs[i]
                    rbc = ps_u.tile([64, TT], f32, tag="u", name="rbc")
                    nc.tensor.matmul(
                        rbc[:], ones_b[64:65, 0:64], osb[64:65, :],
                        start=True, stop=True,
                    )
                    rbs = stage.tile([64, TT], f32, tag="rbs", name="rbs")
                    nc.vector.reciprocal_approx_fast(rbs[:], rbc[:])
                    if i == 0:
                        nc.vector.tensor_tensor(
                            oT[0:64, p, bass.ts(j, TT)], osb[0:64, :], rbs[:],
                            mybir.AluOpType.mult,
                        )
                    else:
                        onrm = stage.tile([64, TT], bf16, tag="onrm",
                                          name="onrm")
                        nc.vector.tensor_tensor(
                            onrm[:], osb[0:64, :], rbs[:], mybir.AluOpType.mult
                        )
                        nc.sync.dma_start(
                            oT[64:128, p, bass.ts(j, TT)], onrm[:]
                        )

            # scores PSUM: two parity tiles of 2 banks each.  Separate tiles
            # (not one [P,4,TT] tensor) so the tile-granular WAR dependency
            # lets sc(kc+2) overlap exp(kc+1): one tile would serialize every
            # score matmul behind the latest exp read, collapsing the
            # pipeline to 1-deep (measured 1.66us/kc vs ACT's 1.11us).
            big_scs = [ps_sc.tile([P, 2, TT], f32, tag=f"sc{par}",
                                  name=f"sc{par}") for par in range(2)]

            # ---- attention: ACT-paced kc pipeline with PE fillers.  The
            # next block's first two score pairs are emitted during the
            # current block's last two kc slots so the exp stream crosses
            # block boundaries without a bubble. ----
            blocks = [(j, p) for p in range(2) for j in range(NJ)]

            def sc_emit_b(t, kc):
                j, p = blocks[t]
                ensure(("k", kc // 4, p))
                ensure(("q", j, p))
                sc = big_scs[kc % 2]
                for i in range(2):
                    lo, hi = 64 * i, 64 * i + 64
                    nc.tensor.matmul(
                        sc[:, i, :],
                        kT[lo:hi, p, bass.ts(kc, P)],
                        qT[lo:hi, p, bass.ts(j, TT)],
                        start=True, stop=True,
                    )

            pending_norm = None
            for t, (j, p) in enumerate(blocks):
                o_ps = ps_acc.tile([P, 2, TT], f32, tag="acc", name="o_ps")

                def av_emit(kc, ex, p=p, o_ps=o_ps):
                    ensure(("v", kc // 4, kc % 4))
                    for i in range(2):
                        h = 2 * p + i
                        nc.tensor.matmul(
                            o_ps[0:VW, i, :],
                            vt[:, kc, bass.ds(VW * h, VW)],
                            ex[:, i, :],
                            start=(kc == 0), stop=(kc == NKT - 1),
                        )

                if t == 0:
                    sc_emit_b(0, 0)
                    sc_emit_b(0, 1)
                if pending_norm is not None:
                    pending_norm()
                    pending_norm = None
                ndrain = 3 if t == 0 else (2 if t < NJ else 1)
                prev = None
                for kc in range(NKT):
                    ex = exps.tile([P, 2, TT], bf16, tag="exp", name="ex")
                    nc.scalar.activation(
                        ex[:], big_scs[kc % 2][:], Exp, scale=0.125,
                    )
                    drain_filler(NKT - kc, ndrain)
                    if prev is not None:
                        av_emit(kc - 1, prev)
                    if kc + 2 < NKT:
                        sc_emit_b(t, kc + 2)
                    elif t + 1 < len(blocks):
                        sc_emit_b(t + 1, kc - (NKT - 2))
                    prev = ex
                av_emit(NKT - 1, prev)
                osbs = norm_copies(o_ps)
                pending_norm = (
                    lambda j=j, p=p, osbs=osbs: norm_arith(j, p, osbs)
                )
                if p == 1:
                    for oc in range(D // P):
                        fillers.append((("o", j, oc), outproj_group(j, oc)))
            pending_norm()
            while fillers or active:
                drain_filler(NKT)

    nc.compile()
    return nc


def make_in_maps(x, Wq, bq, Wk, bk, Wv, Wo):
    import ml_dtypes

    bf = ml_dtypes.bfloat16
    xt = [np.ascontiguousarray(x[b].T.astype(bf)) for b in range(B)]

    in_maps = []
    for c in range(8):
        b, g = divmod(c, 4)
        cs = slice(COLS * g, COLS * (g + 1))
        in_maps.append({
            "xt": xt[b],
            "wq": np.ascontiguousarray(Wq[:, cs].astype(bf)),
            "wk": np.ascontiguousarray(Wk[:, cs].astype(bf)),
            "wv": np.ascontiguousarray(Wv[:, cs].astype(bf)),
            "wo": np.ascontiguousarray(Wo[cs, :].astype(bf)),
            "bq": np.ascontiguousarray(bq[cs].reshape(2, P).T),
            "bk": np.ascontiguousarray(bk[cs].reshape(2, P).T),
        })
    return in_maps


def kernel(x, Wq, bq, Wk, bk, Wv, bv, Wo, bo):
    from concourse import bass_utils

    x = np.asarray(x, dtype=np.float32)
    Wq = np.asarray(Wq, dtype=np.float32)
    Wk = np.asarray(Wk, dtype=np.float32)
    Wv = np.asarray(Wv, dtype=np.float32)
    Wo = np.asarray(Wo, dtype=np.float32)
    bq = np.asarray(bq, dtype=np.float32)
    bk = np.asarray(bk, dtype=np.float32)
    bv = np.asarray(bv, dtype=np.float32)
    bo = np.asarray(bo, dtype=np.float32)

    if "nc" not in _CACHE:
        _CACHE["nc"] = _build()
    nc = _CACHE["nc"]

    in_maps = make_in_maps(x, Wq, bq, Wk, bk, Wv, Wo)
    res = bass_utils.run_bass_kernel_spmd(nc, in_maps, core_ids=list(range(8)))

    out = np.zeros((B, S, D), dtype=np.float32)
    for c in range(8):
        out[c // 4] += res.results[c]["out_t"].T.astype(np.float32)
    out += bo + bv @ Wo
    return out
